# revision 7
# baseline (speedup 1.0000x reference)
"""Batched causal self-attention (B=4, T=2048, C=1024, H=16) on 8 trn2 NeuronCores.

Sharding: data-parallel over B (4) x tensor-parallel over head-halves (2).
Core c handles batch b=c//2, heads [hh*8, hh*8+8) with hh=c%2. Each core
computes its qkv projection slice, causal attention for its 8 heads, and a
partial output projection (512 rows of W_proj); the host sums the two
partials per batch (the TP all-reduce).

v2: chunk-major schedule. The kernel is PE-streaming-bound (~225us of
matmul columns at 2.4GHz); everything else is arranged to keep the PE
continuously fed:
  - startup: k-outer qkv projection waves paced by the xT tile DMAs, so the
    PE starts ~4us in instead of waiting for the full 4MB activation load.
  - chunk-major attention (for c: for pair:) so the output projection for
    chunk c-1 is ready to interleave during chunk c instead of piling up
    behind the last pair (pair-major left an 8us serialized tail).
  - superslots of two key blocks: S^T via two K=64 matmuls per head pair
    (PE row groups), exp on ACT (scale=1/8 folded, no max-subtraction:
    scores ~N(0,0.4^2)), causal mask as an in-place [128,128] multiply on
    only the diagonal block (the rest of the band needs no mask), AV psum
    accumulation with v_aug (64 v-dims + ones column for the denominator +
    zero pad to 128 for FWL) running 2 superslots behind S.
  - independent PE work (next chunk's qkv projection slices, v tiles, and
    chunk c-1's output projection) is emitted into the superslot stream via
    a paced backlog to cover the exp shadow.
  - DMA triggers cost ~0.7us each on the sync queue, so inputs are
    consolidated (wqk packed host-side into one tensor, 2 triggers) and
    outputs merged to one [128,1024] DMA per row tile.
"""

import numpy as np
import ml_dtypes

import concourse.bass as bass
import concourse.mybir as mybir
import concourse.tile as tile
from concourse import bacc
from concourse.bass import ds, ts
from concourse.bass_utils import run_bass_kernel_spmd

B, T, C, H = 4, 2048, 1024, 16
D = 64
NCORES = 8
NPAIR = 4              # head pairs per core (8 heads)
NK = C // 128          # 8 contraction tiles over C
NT = T // 128          # 16 tiles over T
NCH = T // 512         # 4 query chunks
INV_SCALE = 0.125      # 1 / sqrt(C // H)

f32 = mybir.dt.float32
bf16 = mybir.dt.bfloat16
fp16 = mybir.dt.float16

_cache = {}
LAST_RESULTS = None    # test harness reads exec_time_ns from here

# If the caller sets BASS_TRACE=1, run_bass_kernel_spmd imports
# antenv.axon_hooks, which some container images don't ship. Provide a stub
# so tracing degrades gracefully instead of raising ImportError.
try:
    import antenv.axon_hooks  # noqa: F401
except ImportError:
    import sys as _sys
    import types as _types

    _m = _types.ModuleType("antenv.axon_hooks")
    _m._hook = None
    _m.set_axon_ntff_profile_hook = lambda h: setattr(_m, "_hook", h)
    _m.get_axon_ntff_profile_hook = lambda: _m._hook
    _sys.modules["antenv.axon_hooks"] = _m


def _build():
    nc = bacc.Bacc("TRN2", target_bir_lowering=False, debug=False)
    xT_d = nc.dram_tensor("xT", [C, T], bf16, kind="ExternalInput").ap()
    # host-packed: [p, ko, pair*2+m2 flattened to 1024 cols]
    wqk_d = nc.dram_tensor("wqk", [128, NK, NPAIR * 256], bf16,
                           kind="ExternalInput").ap()
    wv_d = nc.dram_tensor("wv", [C, 512], bf16, kind="ExternalInput").ap()
    wp_d = nc.dram_tensor("wp", [512, C], fp16, kind="ExternalInput").ap()
    mask_d = nc.dram_tensor("mask", [128, 512], bf16, kind="ExternalInput").ap()
    out_d = nc.dram_tensor("out", [T, C], f32, kind="ExternalOutput").ap()

    Exp = mybir.ActivationFunctionType.Exp

    with tile.TileContext(nc) as tc:
        with tc.tile_pool(name="persist", bufs=1) as persist:
            xT_t = persist.tile([128, NK, T], bf16, tag="xT")
            wqk_t = persist.tile([128, NK, NPAIR * 256], bf16, tag="wqk")
            wv_t = persist.tile([128, NK, 512], bf16, tag="wv")
            wp_t = persist.tile([128, 4, C], fp16, tag="wp")
            mask_t = persist.tile([128, 512], bf16, tag="mask")
            # per-head blocks padded to 128 cols (v[0:64] | ones at 64 | zeros)
            # so the AV matmul's weight load is exactly 128 columns -> FWL.
            v_aug = persist.tile([128, NT, 8 * 128], bf16, tag="vaug")
            qk = persist.tile([128, NPAIR, 2, T], bf16, tag="qk")
            yT = persist.tile([128, NPAIR, T], fp16, tag="yT")

            # ---- input DMAs, ordered by first use. The startup waves only
            # ---- stream query-chunk-0 columns of xT, so xT is split: cols
            # ---- [0,512) per k-tile early (paces wave A), the rest after wv.
            for i in range(4):
                nc.sync.dma_start(wqk_t[:, 2 * i:2 * i + 2],
                                  wqk_d[:, 2 * i:2 * i + 2])
                for k in (2 * i, 2 * i + 1):
                    nc.sync.dma_start(xT_t[:, k, 0:512],
                                      xT_d[ds(k * 128, 128), 0:512])
            nc.sync.dma_start(
                wv_t[:], wv_d.rearrange("(k p) m -> p k m", p=128))
            for k in range(NK):
                nc.sync.dma_start(xT_t[:, k, 512:T],
                                  xT_d[ds(k * 128, 128), 512:T])
            nc.sync.dma_start(mask_t[:], mask_d)
            nc.sync.dma_start(
                wp_t[:], wp_d.rearrange("(kp p) m -> p kp m", p=128))

            # v_aug pad init on DVE (idle during the DMA ramp): zeros in
            # cols [D,128) of each head block, ones column at D.
            va4 = v_aug.rearrange("p n (h e) -> p n h e", e=128)
            nc.vector.memset(va4[:, :, :, D:128].bitcast(mybir.dt.uint16), 0)
            nc.vector.memset(
                va4[:, :, :, D:D + 1].bitcast(mybir.dt.uint16), 0x3F80)

            # ---- startup waves: qk projection for chunk 0 (k-outer, DMA
            # ---- paced), then v tiles 0..3.
            with tc.tile_pool(name="spool", bufs=8, space="PSUM") as spool:
                wave = [
                    spool.tile([128, 512], f32, tag="w", name=f"waveA{g}")
                    for g in range(8)
                ]
                for k in range(NK):
                    for g in range(8):
                        p, m2 = g // 2, g % 2
                        nc.tensor.matmul(
                            wave[g][:],
                            wqk_t[:, k, ds(p * 256 + m2 * 128, 128)],
                            xT_t[:, k, 0:512],
                            start=(k == 0), stop=(k == NK - 1))
                for g in range(8):
                    p, m2 = g // 2, g % 2
                    nc.scalar.copy(qk[:, p, m2, 0:512], wave[g][:])
                vwave = [
                    spool.tile([128, 512], f32, tag="w", name=f"waveV{t}")
                    for t in range(4)
                ]
                for k in range(NK):
                    for t in range(4):
                        nc.tensor.matmul(
                            vwave[t][:], xT_t[:, k, ts(t, 128)], wv_t[:, k],
                            start=(k == 0), stop=(k == NK - 1))
                for t in range(4):
                    nc.vector.tensor_copy(
                        va4[:, t, :, 0:D],
                        vwave[t].rearrange("p (h d) -> p h d", d=D))

            with (
                tc.tile_pool(name="epool", bufs=6) as epool,
                tc.tile_pool(name="npool", bufs=2) as npool,
                tc.tile_pool(name="opool", bufs=2) as opool,
                tc.tile_pool(name="spsum", bufs=3, space="PSUM") as spsum,
                tc.tile_pool(name="avpsum", bufs=2, space="PSUM") as avpsum,
            ):
                # ---- interleavable PE work units ----
                def qkproj_group(p, m2, n):
                    def go():
                        ps = spsum.tile([128, 512], f32, tag="s",
                                        name=f"qkps{p}_{m2}_{n}")
                        for k in range(NK):
                            nc.tensor.matmul(
                                ps[:], wqk_t[:, k, ds(p * 256 + m2 * 128, 128)],
                                xT_t[:, k, ds(n * 512, 512)],
                                start=(k == 0), stop=(k == NK - 1))
                        # chunks 0/1 leave ACT mostly idle; drain there
                        if n <= 2:
                            nc.scalar.copy(qk[:, p, m2, ds(n * 512, 512)], ps[:])
                        else:
                            nc.vector.tensor_copy(
                                qk[:, p, m2, ds(n * 512, 512)], ps[:])
                    return go

                def vproj_group(t):
                    def go():
                        ps = spsum.tile([128, 512], f32, tag="s",
                                        name=f"vps{t}")
                        for k in range(NK):
                            nc.tensor.matmul(
                                ps[:], xT_t[:, k, ts(t, 128)], wv_t[:, k],
                                start=(k == 0), stop=(k == NK - 1))
                        nc.vector.tensor_copy(
                            va4[:, t, :, 0:D],
                            ps.rearrange("p (h d) -> p h d", d=D))
                    return go

                def proj_group(t):
                    def go():
                        o_t = opool.tile([128, 1024], f32, tag="o",
                                         name=f"o{t}")
                        ps = spsum.tile([128, 1024], f32, tag="s",
                                        name=f"pps{t}")
                        for n2 in range(2):
                            for kp in range(4):
                                nc.tensor.matmul(
                                    ps[:, ds(n2 * 512, 512)],
                                    yT[:, kp, ts(t, 128)],
                                    wp_t[:, kp, ds(n2 * 512, 512)],
                                    start=(kp == 0), stop=(kp == 3))
                        nc.vector.tensor_copy(o_t[:], ps[:])
                        nc.sync.dma_start(out_d[ds(t * 128, 128)], o_t[:])
                    return go

                deferred_norm = []

                def make_norm(avs, head, p, c):
                    def go():
                        dn = npool.tile([1, 512], f32, tag="dn",
                                        name=f"dn{p}_{c}_{head}")
                        nc.vector.tensor_copy(dn[:], avs[D:D + 1, :])
                        rb = npool.tile([D, 512], f32, tag="rb",
                                        name=f"rb{p}_{c}_{head}")
                        nc.gpsimd.partition_broadcast(rb[:], dn[:])
                        rr = npool.tile([D, 512], f32, tag="rr",
                                        name=f"rr{p}_{c}_{head}")
                        nc.vector.reciprocal_approx_fast(out=rr[:], in_=rb[:])
                        nc.vector.tensor_mul(
                            yT[ds(D * head, D), p, ds(c * 512, 512)],
                            avs[0:D, :], rr[:])
                    return go

                # ---- attention: chunk-major with PE backlog interleave ----
                for c in range(NCH):
                    backlog = []
                    if c + 1 < NCH:
                        for t in range(4 * (c + 1), 4 * (c + 2)):
                            backlog.append(vproj_group(t))
                        for g in range(8):
                            backlog.append(qkproj_group(g // 2, g % 2, c + 1))
                    # output projection for chunk X interleaves two chunks
                    # later (X+2), where its yT norms are long complete; the
                    # ACT-saturated final chunk gets proj(c1) ungated plus
                    # proj(c2) in a second queue paced only after the
                    # (p=0, s==1) norm drain.
                    backlog2 = []
                    if c == 2:
                        for t in range(0, 4):
                            backlog.append(proj_group(t))
                    if c == 3:
                        for t in range(4, 8):
                            backlog.append(proj_group(t))
                        for t in range(8, 12):
                            backlog2.append(proj_group(t))
                    nblk = 4 * (c + 1)
                    nsuper = nblk // 2
                    total_slots = 4 * (nsuper + 2)
                    done_slots = 0
                    emitted = 0
                    emitted2 = 0
                    for p in range(NPAIR):
                        av_A = avpsum.tile([128, 512], f32, tag="av",
                                           name=f"avA{p}_{c}")
                        av_B = avpsum.tile([128, 512], f32, tag="av",
                                           name=f"avB{p}_{c}")
                        pend = {}
                        for s in range(nsuper + 2):
                            if s == 1 and deferred_norm:
                                for fn in deferred_norm:
                                    fn()
                                deferred_norm.clear()
                            if s < nsuper:
                                sA = spsum.tile([128, 1024], f32, tag="s",
                                                name=f"sA{p}_{c}_{s}")
                                sB = spsum.tile([128, 1024], f32, tag="s",
                                                name=f"sB{p}_{c}_{s}")
                                i = s - (nsuper - 2)
                                if i < 0:
                                    for half in (0, 1):
                                        tj = 2 * s + half
                                        nc.tensor.matmul(
                                            sA[:, ds(half * 512, 512)],
                                            qk[0:D, p, 1, ts(tj, 128)],
                                            qk[0:D, p, 0, ds(c * 512, 512)],
                                            start=True, stop=True)
                                        nc.tensor.matmul(
                                            sB[:, ds(half * 512, 512)],
                                            qk[D:128, p, 1, ts(tj, 128)],
                                            qk[D:128, p, 0, ds(c * 512, 512)],
                                            start=True, stop=True)
                                else:
                                    # band: columns [0, 128d) are fully masked
                                    # — compute only the live range
                                    for half in (0, 1):
                                        tj = 2 * s + half
                                        off = 128 * (2 * i + half)
                                        w = 512 - off
                                        nc.tensor.matmul(
                                            sA[:, ds(half * 512 + off, w)],
                                            qk[0:D, p, 1, ts(tj, 128)],
                                            qk[0:D, p, 0, ds(c * 512 + off, w)],
                                            start=True, stop=True)
                                        nc.tensor.matmul(
                                            sB[:, ds(half * 512 + off, w)],
                                            qk[D:128, p, 1, ts(tj, 128)],
                                            qk[D:128, p, 0, ds(c * 512 + off, w)],
                                            start=True, stop=True)
                                e_A = epool.tile([128, 1024], bf16, tag="e",
                                                 name=f"eA{p}_{c}_{s}")
                                e_B = epool.tile([128, 1024], bf16, tag="e",
                                                 name=f"eB{p}_{c}_{s}")
                                if i >= 0:
                                    for half in (0, 1):
                                        off = 128 * (2 * i + half)
                                        w = 512 - off
                                        sl = ds(half * 512 + off, w)
                                        nc.scalar.activation(e_A[:, sl], sA[:, sl],
                                                             Exp, scale=INV_SCALE)
                                        nc.scalar.activation(e_B[:, sl], sB[:, sl],
                                                             Exp, scale=INV_SCALE)
                                        # only the diagonal 128 columns of the
                                        # live range contain masked entries;
                                        # zero them with an in-place triangle
                                        # multiply.
                                        sd = ds(half * 512 + off, 128)
                                        nc.gpsimd.tensor_mul(
                                            e_A[:, sd], e_A[:, sd], mask_t[:, 0:128])
                                        nc.gpsimd.tensor_mul(
                                            e_B[:, sd], e_B[:, sd], mask_t[:, 0:128])
                                else:
                                    nc.scalar.activation(e_A[:], sA[:], Exp,
                                                         scale=INV_SCALE)
                                    nc.scalar.activation(e_B[:], sB[:], Exp,
                                                         scale=INV_SCALE)
                                pend[s] = (e_A, e_B)
                            if s >= 2:
                                e_A, e_B = pend.pop(s - 2)
                                s2 = s - 2
                                for half in (0, 1):
                                    tj = 2 * s2 + half
                                    dd = tj - (nblk - 4)
                                    # masked band columns [0,128d) of E are
                                    # zero after the mask multiply — skip them
                                    off = 128 * dd if dd > 0 else 0
                                    w = 512 - off
                                    nc.tensor.matmul(
                                        av_A[:, ds(off, w)],
                                        v_aug[:, tj, ds(2 * p * 128, 128)],
                                        e_A[:, ds(half * 512 + off, w)],
                                        start=(tj == 0), stop=(tj == nblk - 1))
                                    nc.tensor.matmul(
                                        av_B[:, ds(off, w)],
                                        v_aug[:, tj, ds((2 * p + 1) * 128, 128)],
                                        e_B[:, ds(half * 512 + off, w)],
                                        start=(tj == 0), stop=(tj == nblk - 1))
                            # pace the backlog across the chunk's superslots
                            done_slots += 1
                            want = -(-len(backlog) * done_slots // total_slots)
                            while emitted < want and emitted < len(backlog):
                                backlog[emitted]()
                                emitted += 1
                            if done_slots >= 2 and backlog2:
                                want2 = -(-len(backlog2) * (done_slots - 1)
                                          // (total_slots - 1))
                                while emitted2 < want2 and emitted2 < len(backlog2):
                                    backlog2[emitted2]()
                                    emitted2 += 1
                        # stage av to SBUF with one copy so the PSUM banks free
                        # early; normalize from the staged copy.
                        for head, av in ((0, av_A), (1, av_B)):
                            avs = npool.tile([D + 1, 512], f32, tag="avs",
                                             bufs=4, name=f"avs{p}_{c}_{head}")
                            nc.vector.tensor_copy(avs[:], av[0:D + 1, :])
                            deferred_norm.append(make_norm(avs, head, p, c))

                for fn in deferred_norm:
                    fn()
                deferred_norm.clear()

                # ---- projection tail (last query chunk) ----
                for t in range(12, NT):
                    proj_group(t)()

    nc.compile()
    return nc


def _make_mask():
    # mask[p, j] = 1 iff j >= p: causal triangle in the first 128 cols of a
    # band live range, ones beyond.
    p = np.arange(128)[:, None]
    j = np.arange(512)[None, :]
    return (j >= p).astype(ml_dtypes.bfloat16)


def kernel(x: np.ndarray, W_attn: np.ndarray, W_proj: np.ndarray) -> np.ndarray:
    global LAST_RESULTS
    x = np.asarray(x, dtype=np.float32)
    W_attn = np.asarray(W_attn, dtype=np.float32)
    W_proj = np.asarray(W_proj, dtype=np.float32)

    nc = _cache.get("nc")
    if nc is None:
        nc = _build()
        _cache["nc"] = nc

    mask = _make_mask()
    xTs = [np.ascontiguousarray(x[b].T).astype(ml_dtypes.bfloat16) for b in range(B)]
    in_maps = []
    for cid in range(NCORES):
        b, hh = cid // 2, cid % 2
        qcols = W_attn[:, hh * 512:(hh + 1) * 512]
        kcols = W_attn[:, C + hh * 512:C + (hh + 1) * 512]
        wqk = np.concatenate([qcols, kcols], axis=1)                  # [1024, 1024]
        # pack to [p, ko, pair, m2, mm] -> [128, NK, 1024]
        wqk_pack = np.ascontiguousarray(
            wqk.reshape(NK, 128, 2, NPAIR, 128).transpose(1, 0, 3, 2, 4)
            .reshape(128, NK, NPAIR * 256)
        ).astype(ml_dtypes.bfloat16)
        wv = np.ascontiguousarray(
            W_attn[:, 2 * C + hh * 512:2 * C + (hh + 1) * 512]
        ).astype(ml_dtypes.bfloat16)
        wp = np.ascontiguousarray(W_proj[hh * 512:(hh + 1) * 512, :]).astype(np.float16)
        in_maps.append({
            "xT": xTs[b], "wqk": wqk_pack, "wv": wv, "wp": wp, "mask": mask,
        })

    res = run_bass_kernel_spmd(nc, in_maps, core_ids=list(range(NCORES)))
    LAST_RESULTS = res
    parts = [res.results[cid]["out"] for cid in range(NCORES)]
    out = np.stack([parts[2 * b] + parts[2 * b + 1] for b in range(B)], axis=0)
    return np.ascontiguousarray(out, dtype=np.float32)


# revision 8
# speedup vs baseline: 1.7528x; 1.7528x over previous
"""Batched causal self-attention (B=4, T=2048, C=1024, H=16) on 8 trn2 NeuronCores.

Sharding: data-parallel over B (4) x tensor-parallel over head-halves (2).
Core c handles batch b=c//2, heads [hh*8, hh*8+8) with hh=c%2. Each core
computes its qkv projection slice, causal attention for its 8 heads, and a
partial output projection (512 rows of W_proj); the host sums the two
partials per batch (the TP all-reduce).

v2: chunk-major schedule. The kernel is PE-streaming-bound (~225us of
matmul columns at 2.4GHz); everything else is arranged to keep the PE
continuously fed:
  - startup: k-outer qkv projection waves paced by the xT tile DMAs, so the
    PE starts ~4us in instead of waiting for the full 4MB activation load.
  - chunk-major attention (for c: for pair:) so the output projection for
    chunk c-1 is ready to interleave during chunk c instead of piling up
    behind the last pair (pair-major left an 8us serialized tail).
  - superslots of two key blocks: S^T via two K=64 matmuls per head pair
    (PE row groups), exp on ACT (scale=1/8 folded, no max-subtraction:
    scores ~N(0,0.4^2)), causal mask as an in-place [128,128] multiply on
    only the diagonal block (the rest of the band needs no mask), AV psum
    accumulation with v_aug (64 v-dims + ones column for the denominator +
    zero pad to 128 for FWL) running 2 superslots behind S.
  - independent PE work (next chunk's qkv projection slices, v tiles, and
    chunk c-1's output projection) is emitted into the superslot stream via
    a paced backlog to cover the exp shadow.
  - DMA triggers cost ~0.7us each on the sync queue, so inputs are
    consolidated (wqk packed host-side into one tensor, 2 triggers) and
    outputs merged to one [128,1024] DMA per row tile.
"""

import numpy as np
import ml_dtypes

import concourse.bass as bass
import concourse.mybir as mybir
import concourse.tile as tile
from concourse import bacc
from concourse.bass import ds, ts
from concourse.bass_utils import run_bass_kernel_spmd

B, T, C, H = 4, 2048, 1024, 16
D = 64
NCORES = 8
NPAIR = 4              # head pairs per core (8 heads)
NK = C // 128          # 8 contraction tiles over C
NT = T // 128          # 16 tiles over T
NCH = T // 512         # 4 query chunks
INV_SCALE = 0.125      # 1 / sqrt(C // H)

f32 = mybir.dt.float32
bf16 = mybir.dt.bfloat16
fp16 = mybir.dt.float16

_cache = {}
LAST_RESULTS = None    # test harness reads exec_time_ns from here

# If the caller sets BASS_TRACE=1, run_bass_kernel_spmd imports
# antenv.axon_hooks, which some container images don't ship. Provide a stub
# so tracing degrades gracefully instead of raising ImportError.
try:
    import antenv.axon_hooks  # noqa: F401
except ImportError:
    import sys as _sys
    import types as _types

    _m = _types.ModuleType("antenv.axon_hooks")
    _m._hook = None
    _m.set_axon_ntff_profile_hook = lambda h: setattr(_m, "_hook", h)
    _m.get_axon_ntff_profile_hook = lambda: _m._hook
    _sys.modules["antenv.axon_hooks"] = _m


def _build():
    nc = bacc.Bacc("TRN2", target_bir_lowering=False, debug=False)
    xT_d = nc.dram_tensor("xT", [C, T], bf16, kind="ExternalInput").ap()
    # host-packed: [p, ko, pair*2+m2 flattened to 1024 cols]
    wqk_d = nc.dram_tensor("wqk", [128, NK, NPAIR * 256], bf16,
                           kind="ExternalInput").ap()
    wv_d = nc.dram_tensor("wv", [C, 512], bf16, kind="ExternalInput").ap()
    wp_d = nc.dram_tensor("wp", [512, C], fp16, kind="ExternalInput").ap()
    mask_d = nc.dram_tensor("mask", [128, 512], bf16, kind="ExternalInput").ap()
    out_d = nc.dram_tensor("out", [T, C], f32, kind="ExternalOutput").ap()

    Exp = mybir.ActivationFunctionType.Exp

    with tile.TileContext(nc) as tc:
        with tc.tile_pool(name="persist", bufs=1) as persist:
            xT_t = persist.tile([128, NK, T], bf16, tag="xT")
            wqk_t = persist.tile([128, NK, NPAIR * 256], bf16, tag="wqk")
            wv_t = persist.tile([128, NK, 512], bf16, tag="wv")
            wp_t = persist.tile([128, 4, C], fp16, tag="wp")
            mask_t = persist.tile([128, 512], bf16, tag="mask")
            # per-head blocks padded to 128 cols (v[0:64] | ones at 64 | zeros)
            # so the AV matmul's weight load is exactly 128 columns -> FWL.
            v_aug = persist.tile([128, NT, 8 * 128], bf16, tag="vaug")
            qk = persist.tile([128, NPAIR, 2, T], bf16, tag="qk")
            yT = persist.tile([128, NPAIR, T], fp16, tag="yT")

            # ---- input DMAs, ordered by first use. The startup waves only
            # ---- stream query-chunk-0 columns of xT, so xT is split: cols
            # ---- [0,512) per k-tile early (paces wave A), the rest after wv.
            for i in range(4):
                nc.sync.dma_start(wqk_t[:, 2 * i:2 * i + 2],
                                  wqk_d[:, 2 * i:2 * i + 2])
                for k in (2 * i, 2 * i + 1):
                    nc.sync.dma_start(xT_t[:, k, 0:512],
                                      xT_d[ds(k * 128, 128), 0:512])
            nc.sync.dma_start(
                wv_t[:], wv_d.rearrange("(k p) m -> p k m", p=128))
            for k in range(NK):
                nc.sync.dma_start(xT_t[:, k, 512:T],
                                  xT_d[ds(k * 128, 128), 512:T])
            nc.sync.dma_start(mask_t[:], mask_d)
            nc.sync.dma_start(
                wp_t[:], wp_d.rearrange("(kp p) m -> p kp m", p=128))

            # v_aug pad init on DVE (idle during the DMA ramp): zeros in
            # cols [D,128) of each head block, ones column at D.
            va4 = v_aug.rearrange("p n (h e) -> p n h e", e=128)
            nc.vector.memset(va4[:, :, :, D:128].bitcast(mybir.dt.uint16), 0)
            nc.vector.memset(
                va4[:, :, :, D:D + 1].bitcast(mybir.dt.uint16), 0x3F80)

            # ---- startup waves: qk projection for chunk 0 (k-outer, DMA
            # ---- paced), then v tiles 0..3.
            with tc.tile_pool(name="spool", bufs=8, space="PSUM") as spool:
                wave = [
                    spool.tile([128, 512], f32, tag="w", name=f"waveA{g}")
                    for g in range(8)
                ]
                for k in range(NK):
                    for g in range(8):
                        p, m2 = g // 2, g % 2
                        nc.tensor.matmul(
                            wave[g][:],
                            wqk_t[:, k, ds(p * 256 + m2 * 128, 128)],
                            xT_t[:, k, 0:512],
                            start=(k == 0), stop=(k == NK - 1))
                for g in range(8):
                    p, m2 = g // 2, g % 2
                    nc.scalar.copy(qk[:, p, m2, 0:512], wave[g][:])
                vwave = [
                    spool.tile([128, 512], f32, tag="w", name=f"waveV{t}")
                    for t in range(4)
                ]
                for k in range(NK):
                    for t in range(4):
                        nc.tensor.matmul(
                            vwave[t][:], xT_t[:, k, ts(t, 128)], wv_t[:, k],
                            start=(k == 0), stop=(k == NK - 1))
                for t in range(4):
                    nc.vector.tensor_copy(
                        va4[:, t, :, 0:D],
                        vwave[t].rearrange("p (h d) -> p h d", d=D))

            with (
                tc.tile_pool(name="epool", bufs=6) as epool,
                tc.tile_pool(name="npool", bufs=2) as npool,
                tc.tile_pool(name="opool", bufs=2) as opool,
                tc.tile_pool(name="spsum", bufs=3, space="PSUM") as spsum,
                tc.tile_pool(name="avpsum", bufs=2, space="PSUM") as avpsum,
            ):
                # ---- interleavable PE work units ----
                def qkproj_group(p, m2, n):
                    def go():
                        ps = spsum.tile([128, 512], f32, tag="s",
                                        name=f"qkps{p}_{m2}_{n}")
                        for k in range(NK):
                            nc.tensor.matmul(
                                ps[:], wqk_t[:, k, ds(p * 256 + m2 * 128, 128)],
                                xT_t[:, k, ds(n * 512, 512)],
                                start=(k == 0), stop=(k == NK - 1))
                        # chunks 0/1 leave ACT mostly idle; drain there
                        if n <= 2:
                            nc.scalar.copy(qk[:, p, m2, ds(n * 512, 512)], ps[:])
                        else:
                            nc.vector.tensor_copy(
                                qk[:, p, m2, ds(n * 512, 512)], ps[:])
                    return go

                def vproj_group(t):
                    def go():
                        ps = spsum.tile([128, 512], f32, tag="s",
                                        name=f"vps{t}")
                        for k in range(NK):
                            nc.tensor.matmul(
                                ps[:], xT_t[:, k, ts(t, 128)], wv_t[:, k],
                                start=(k == 0), stop=(k == NK - 1))
                        nc.vector.tensor_copy(
                            va4[:, t, :, 0:D],
                            ps.rearrange("p (h d) -> p h d", d=D))
                    return go

                def proj_group(t):
                    def go():
                        o_t = opool.tile([128, 1024], f32, tag="o",
                                         name=f"o{t}")
                        ps = spsum.tile([128, 1024], f32, tag="s",
                                        name=f"pps{t}")
                        for n2 in range(2):
                            for kp in range(4):
                                nc.tensor.matmul(
                                    ps[:, ds(n2 * 512, 512)],
                                    yT[:, kp, ts(t, 128)],
                                    wp_t[:, kp, ds(n2 * 512, 512)],
                                    start=(kp == 0), stop=(kp == 3))
                        nc.vector.tensor_copy(o_t[:], ps[:])
                        nc.sync.dma_start(out_d[ds(t * 128, 128)], o_t[:])
                    return go

                deferred_norm = []

                def make_norm(avs, head, p, c):
                    def go():
                        dn = npool.tile([1, 512], f32, tag="dn",
                                        name=f"dn{p}_{c}_{head}")
                        nc.vector.tensor_copy(dn[:], avs[D:D + 1, :])
                        rb = npool.tile([D, 512], f32, tag="rb",
                                        name=f"rb{p}_{c}_{head}")
                        nc.gpsimd.partition_broadcast(rb[:], dn[:])
                        rr = npool.tile([D, 512], f32, tag="rr",
                                        name=f"rr{p}_{c}_{head}")
                        nc.vector.reciprocal_approx_fast(out=rr[:], in_=rb[:])
                        nc.vector.tensor_mul(
                            yT[ds(D * head, D), p, ds(c * 512, 512)],
                            avs[0:D, :], rr[:])
                    return go

                # ---- attention: chunk-major with PE backlog interleave ----
                for c in range(NCH):
                    backlog = []
                    if c + 1 < NCH:
                        for t in range(4 * (c + 1), 4 * (c + 2)):
                            backlog.append(vproj_group(t))
                        for g in range(8):
                            backlog.append(qkproj_group(g // 2, g % 2, c + 1))
                    # output projection for chunk X interleaves two chunks
                    # later (X+2), where its yT norms are long complete; the
                    # ACT-saturated final chunk gets proj(c1) ungated plus
                    # proj(c2) in a second queue paced only after the
                    # (p=0, s==1) norm drain.
                    backlog2 = []
                    if c == 2:
                        for t in range(0, 4):
                            backlog.append(proj_group(t))
                    if c == 3:
                        for t in range(4, 8):
                            backlog.append(proj_group(t))
                        for t in range(8, 12):
                            backlog2.append(proj_group(t))
                    nblk = 4 * (c + 1)
                    nsuper = nblk // 2
                    total_slots = 4 * (nsuper + 2)
                    done_slots = 0
                    emitted = 0
                    emitted2 = 0
                    for p in range(NPAIR):
                        av_A = avpsum.tile([128, 512], f32, tag="av",
                                           name=f"avA{p}_{c}")
                        av_B = avpsum.tile([128, 512], f32, tag="av",
                                           name=f"avB{p}_{c}")
                        pend = {}
                        for s in range(nsuper + 2):
                            if s == 1 and deferred_norm:
                                for fn in deferred_norm:
                                    fn()
                                deferred_norm.clear()
                            if s < nsuper:
                                sA = spsum.tile([128, 1024], f32, tag="s",
                                                name=f"sA{p}_{c}_{s}")
                                sB = spsum.tile([128, 1024], f32, tag="s",
                                                name=f"sB{p}_{c}_{s}")
                                i = s - (nsuper - 2)
                                if i < 0:
                                    for half in (0, 1):
                                        tj = 2 * s + half
                                        nc.tensor.matmul(
                                            sA[:, ds(half * 512, 512)],
                                            qk[0:D, p, 1, ts(tj, 128)],
                                            qk[0:D, p, 0, ds(c * 512, 512)],
                                            start=True, stop=True)
                                        nc.tensor.matmul(
                                            sB[:, ds(half * 512, 512)],
                                            qk[D:128, p, 1, ts(tj, 128)],
                                            qk[D:128, p, 0, ds(c * 512, 512)],
                                            start=True, stop=True)
                                else:
                                    # band: columns [0, 128d) are fully masked
                                    # — compute only the live range
                                    for half in (0, 1):
                                        tj = 2 * s + half
                                        off = 128 * (2 * i + half)
                                        w = 512 - off
                                        nc.tensor.matmul(
                                            sA[:, ds(half * 512 + off, w)],
                                            qk[0:D, p, 1, ts(tj, 128)],
                                            qk[0:D, p, 0, ds(c * 512 + off, w)],
                                            start=True, stop=True)
                                        nc.tensor.matmul(
                                            sB[:, ds(half * 512 + off, w)],
                                            qk[D:128, p, 1, ts(tj, 128)],
                                            qk[D:128, p, 0, ds(c * 512 + off, w)],
                                            start=True, stop=True)
                                e_A = epool.tile([128, 1024], bf16, tag="e",
                                                 name=f"eA{p}_{c}_{s}")
                                e_B = epool.tile([128, 1024], bf16, tag="e",
                                                 name=f"eB{p}_{c}_{s}")
                                if i >= 0:
                                    for half in (0, 1):
                                        off = 128 * (2 * i + half)
                                        w = 512 - off
                                        sl = ds(half * 512 + off, w)
                                        nc.scalar.activation(e_A[:, sl], sA[:, sl],
                                                             Exp, scale=INV_SCALE)
                                        nc.scalar.activation(e_B[:, sl], sB[:, sl],
                                                             Exp, scale=INV_SCALE)
                                        # only the diagonal 128 columns of the
                                        # live range contain masked entries;
                                        # zero them with an in-place triangle
                                        # multiply.
                                        sd = ds(half * 512 + off, 128)
                                        nc.vector.tensor_mul(
                                            e_A[:, sd], e_A[:, sd], mask_t[:, 0:128])
                                        nc.vector.tensor_mul(
                                            e_B[:, sd], e_B[:, sd], mask_t[:, 0:128])
                                else:
                                    nc.scalar.activation(e_A[:], sA[:], Exp,
                                                         scale=INV_SCALE)
                                    nc.scalar.activation(e_B[:], sB[:], Exp,
                                                         scale=INV_SCALE)
                                pend[s] = (e_A, e_B)
                            if s >= 2:
                                e_A, e_B = pend.pop(s - 2)
                                s2 = s - 2
                                for half in (0, 1):
                                    tj = 2 * s2 + half
                                    dd = tj - (nblk - 4)
                                    # masked band columns [0,128d) of E are
                                    # zero after the mask multiply — skip them
                                    off = 128 * dd if dd > 0 else 0
                                    w = 512 - off
                                    nc.tensor.matmul(
                                        av_A[:, ds(off, w)],
                                        v_aug[:, tj, ds(2 * p * 128, 128)],
                                        e_A[:, ds(half * 512 + off, w)],
                                        start=(tj == 0), stop=(tj == nblk - 1))
                                    nc.tensor.matmul(
                                        av_B[:, ds(off, w)],
                                        v_aug[:, tj, ds((2 * p + 1) * 128, 128)],
                                        e_B[:, ds(half * 512 + off, w)],
                                        start=(tj == 0), stop=(tj == nblk - 1))
                            # pace the backlog across the chunk's superslots
                            done_slots += 1
                            want = -(-len(backlog) * done_slots // total_slots)
                            while emitted < want and emitted < len(backlog):
                                backlog[emitted]()
                                emitted += 1
                            if done_slots >= 2 and backlog2:
                                want2 = -(-len(backlog2) * (done_slots - 1)
                                          // (total_slots - 1))
                                while emitted2 < want2 and emitted2 < len(backlog2):
                                    backlog2[emitted2]()
                                    emitted2 += 1
                        # stage av to SBUF with one copy so the PSUM banks free
                        # early; normalize from the staged copy.
                        for head, av in ((0, av_A), (1, av_B)):
                            avs = npool.tile([D + 1, 512], f32, tag="avs",
                                             bufs=4, name=f"avs{p}_{c}_{head}")
                            nc.vector.tensor_copy(avs[:], av[0:D + 1, :])
                            deferred_norm.append(make_norm(avs, head, p, c))

                for fn in deferred_norm:
                    fn()
                deferred_norm.clear()

                # ---- projection tail (last query chunk) ----
                for t in range(12, NT):
                    proj_group(t)()

    nc.compile()
    return nc


def _make_mask():
    # mask[p, j] = 1 iff j >= p: causal triangle in the first 128 cols of a
    # band live range, ones beyond.
    p = np.arange(128)[:, None]
    j = np.arange(512)[None, :]
    return (j >= p).astype(ml_dtypes.bfloat16)


def kernel(x: np.ndarray, W_attn: np.ndarray, W_proj: np.ndarray) -> np.ndarray:
    global LAST_RESULTS
    x = np.asarray(x, dtype=np.float32)
    W_attn = np.asarray(W_attn, dtype=np.float32)
    W_proj = np.asarray(W_proj, dtype=np.float32)

    nc = _cache.get("nc")
    if nc is None:
        nc = _build()
        _cache["nc"] = nc

    mask = _make_mask()
    xTs = [np.ascontiguousarray(x[b].T).astype(ml_dtypes.bfloat16) for b in range(B)]
    in_maps = []
    for cid in range(NCORES):
        b, hh = cid // 2, cid % 2
        qcols = W_attn[:, hh * 512:(hh + 1) * 512]
        kcols = W_attn[:, C + hh * 512:C + (hh + 1) * 512]
        wqk = np.concatenate([qcols, kcols], axis=1)                  # [1024, 1024]
        # pack to [p, ko, pair, m2, mm] -> [128, NK, 1024]
        wqk_pack = np.ascontiguousarray(
            wqk.reshape(NK, 128, 2, NPAIR, 128).transpose(1, 0, 3, 2, 4)
            .reshape(128, NK, NPAIR * 256)
        ).astype(ml_dtypes.bfloat16)
        wv = np.ascontiguousarray(
            W_attn[:, 2 * C + hh * 512:2 * C + (hh + 1) * 512]
        ).astype(ml_dtypes.bfloat16)
        wp = np.ascontiguousarray(W_proj[hh * 512:(hh + 1) * 512, :]).astype(np.float16)
        in_maps.append({
            "xT": xTs[b], "wqk": wqk_pack, "wv": wv, "wp": wp, "mask": mask,
        })

    res = run_bass_kernel_spmd(nc, in_maps, core_ids=list(range(NCORES)))
    LAST_RESULTS = res
    parts = [res.results[cid]["out"] for cid in range(NCORES)]
    out = np.stack([parts[2 * b] + parts[2 * b + 1] for b in range(B)], axis=0)
    return np.ascontiguousarray(out, dtype=np.float32)


# revision 9
# speedup vs baseline: 2.1131x; 1.2055x over previous
"""Batched causal self-attention (B=4, T=2048, C=1024, H=16) on 8 trn2 NeuronCores.

Sharding: data-parallel over B (4) x tensor-parallel over head-halves (2).
Core c handles batch b=c//2, heads [hh*8, hh*8+8) with hh=c%2. Each core
computes its qkv projection slice, causal attention for its 8 heads, and a
partial output projection (512 rows of W_proj); the host sums the two
partials per batch (the TP all-reduce).

v2: chunk-major schedule. The kernel is PE-streaming-bound (~225us of
matmul columns at 2.4GHz); everything else is arranged to keep the PE
continuously fed:
  - startup: k-outer qkv projection waves paced by the xT tile DMAs, so the
    PE starts ~4us in instead of waiting for the full 4MB activation load.
  - chunk-major attention (for c: for pair:) so the output projection for
    chunk c-1 is ready to interleave during chunk c instead of piling up
    behind the last pair (pair-major left an 8us serialized tail).
  - superslots of two key blocks: S^T via two K=64 matmuls per head pair
    (PE row groups), exp on ACT (scale=1/8 folded, no max-subtraction:
    scores ~N(0,0.4^2)), causal mask as an in-place [128,128] multiply on
    only the diagonal block (the rest of the band needs no mask), AV psum
    accumulation with v_aug (64 v-dims + ones column for the denominator +
    zero pad to 128 for FWL) running 2 superslots behind S.
  - independent PE work (next chunk's qkv projection slices, v tiles, and
    chunk c-1's output projection) is emitted into the superslot stream via
    a paced backlog to cover the exp shadow.
  - DMA triggers cost ~0.7us each on the sync queue, so inputs are
    consolidated (wqk packed host-side into one tensor, 2 triggers) and
    outputs merged to one [128,1024] DMA per row tile.
"""

import numpy as np
import ml_dtypes

import concourse.bass as bass
import concourse.mybir as mybir
import concourse.tile as tile
from concourse import bacc
from concourse.bass import ds, ts
from concourse.bass_utils import run_bass_kernel_spmd

B, T, C, H = 4, 2048, 1024, 16
D = 64
NCORES = 8
NPAIR = 4              # head pairs per core (8 heads)
NK = C // 128          # 8 contraction tiles over C
NT = T // 128          # 16 tiles over T
NCH = T // 512         # 4 query chunks
INV_SCALE = 0.125      # 1 / sqrt(C // H)

f32 = mybir.dt.float32
bf16 = mybir.dt.bfloat16
fp16 = mybir.dt.float16

_cache = {}
LAST_RESULTS = None    # test harness reads exec_time_ns from here

# If the caller sets BASS_TRACE=1, run_bass_kernel_spmd imports
# antenv.axon_hooks, which some container images don't ship. Provide a stub
# so tracing degrades gracefully instead of raising ImportError.
try:
    import antenv.axon_hooks  # noqa: F401
except ImportError:
    import sys as _sys
    import types as _types

    _m = _types.ModuleType("antenv.axon_hooks")
    _m._hook = None
    _m.set_axon_ntff_profile_hook = lambda h: setattr(_m, "_hook", h)
    _m.get_axon_ntff_profile_hook = lambda: _m._hook
    _sys.modules["antenv.axon_hooks"] = _m


def _build():
    nc = bacc.Bacc("TRN2", target_bir_lowering=False, debug=False)
    xT_d = nc.dram_tensor("xT", [C, T], bf16, kind="ExternalInput").ap()
    # host-packed: [p, ko, pair*2+m2 flattened to 1024 cols]
    wqk_d = nc.dram_tensor("wqk", [128, NK, NPAIR * 256], bf16,
                           kind="ExternalInput").ap()
    wv_d = nc.dram_tensor("wv", [C, 512], bf16, kind="ExternalInput").ap()
    wp_d = nc.dram_tensor("wp", [512, C], fp16, kind="ExternalInput").ap()
    mask_d = nc.dram_tensor("mask", [128, 512], bf16, kind="ExternalInput").ap()
    out_d = nc.dram_tensor("out", [T, C], f32, kind="ExternalOutput").ap()

    Exp = mybir.ActivationFunctionType.Exp

    with tile.TileContext(nc) as tc:
        with tc.tile_pool(name="persist", bufs=1) as persist:
            xT_t = persist.tile([128, NK, T], bf16, tag="xT")
            wqk_t = persist.tile([128, NK, NPAIR * 256], bf16, tag="wqk")
            wv_t = persist.tile([128, NK, 512], bf16, tag="wv")
            wp_t = persist.tile([128, 4, C], fp16, tag="wp")
            mask_t = persist.tile([128, 512], bf16, tag="mask")
            # per-head blocks padded to 128 cols (v[0:64] | ones at 64 | zeros)
            # so the AV matmul's weight load is exactly 128 columns -> FWL.
            v_aug = persist.tile([128, NT, 8 * 128], bf16, tag="vaug")
            qk = persist.tile([128, NPAIR, 2, T], bf16, tag="qk")
            yT = persist.tile([128, NPAIR, T], fp16, tag="yT")

            # ---- input DMAs, ordered by first use. The startup waves only
            # ---- stream query-chunk-0 columns of xT, so xT is split: cols
            # ---- [0,512) per k-tile early (paces wave A), the rest after wv.
            for i in range(4):
                nc.sync.dma_start(wqk_t[:, 2 * i:2 * i + 2],
                                  wqk_d[:, 2 * i:2 * i + 2])
                for k in (2 * i, 2 * i + 1):
                    nc.sync.dma_start(xT_t[:, k, 0:512],
                                      xT_d[ds(k * 128, 128), 0:512])
            nc.sync.dma_start(
                wv_t[:], wv_d.rearrange("(k p) m -> p k m", p=128))
            for k in range(NK):
                nc.sync.dma_start(xT_t[:, k, 512:T],
                                  xT_d[ds(k * 128, 128), 512:T])
            nc.sync.dma_start(mask_t[:], mask_d)
            nc.sync.dma_start(
                wp_t[:], wp_d.rearrange("(kp p) m -> p kp m", p=128))

            # v_aug pad init on DVE (idle during the DMA ramp): zeros in
            # cols [D,128) of each head block, ones column at D.
            va4 = v_aug.rearrange("p n (h e) -> p n h e", e=128)
            nc.vector.memset(va4[:, :, :, D:128].bitcast(mybir.dt.uint16), 0)
            nc.vector.memset(
                va4[:, :, :, D:D + 1].bitcast(mybir.dt.uint16), 0x3F80)

            # ---- startup waves: qk projection for chunk 0 (k-outer, DMA
            # ---- paced), then v tiles 0..3.
            with tc.tile_pool(name="spool", bufs=8, space="PSUM") as spool:
                wave = [
                    spool.tile([128, 512], f32, tag="w", name=f"waveA{g}")
                    for g in range(8)
                ]
                for k in range(NK):
                    for g in range(8):
                        p, m2 = g // 2, g % 2
                        nc.tensor.matmul(
                            wave[g][:],
                            wqk_t[:, k, ds(p * 256 + m2 * 128, 128)],
                            xT_t[:, k, 0:512],
                            start=(k == 0), stop=(k == NK - 1))
                for g in range(8):
                    p, m2 = g // 2, g % 2
                    nc.scalar.copy(qk[:, p, m2, 0:512], wave[g][:])
                vwave = [
                    spool.tile([128, 512], f32, tag="w", name=f"waveV{t}")
                    for t in range(4)
                ]
                for k in range(NK):
                    for t in range(4):
                        nc.tensor.matmul(
                            vwave[t][:], xT_t[:, k, ts(t, 128)], wv_t[:, k],
                            start=(k == 0), stop=(k == NK - 1))
                for t in range(4):
                    nc.vector.tensor_copy(
                        va4[:, t, :, 0:D],
                        vwave[t].rearrange("p (h d) -> p h d", d=D))

            with (
                tc.tile_pool(name="epool", bufs=6) as epool,
                tc.tile_pool(name="npool", bufs=2) as npool,
                tc.tile_pool(name="opool", bufs=2) as opool,
                tc.tile_pool(name="spsum", bufs=3, space="PSUM") as spsum,
                tc.tile_pool(name="avpsum", bufs=2, space="PSUM") as avpsum,
            ):
                # ---- interleavable PE work units ----
                def qkproj_group(p, m2, n):
                    def go():
                        ps = spsum.tile([128, 512], f32, tag="s",
                                        name=f"qkps{p}_{m2}_{n}")
                        for k in range(NK):
                            nc.tensor.matmul(
                                ps[:], wqk_t[:, k, ds(p * 256 + m2 * 128, 128)],
                                xT_t[:, k, ds(n * 512, 512)],
                                start=(k == 0), stop=(k == NK - 1))
                        nc.vector.tensor_copy(
                            qk[:, p, m2, ds(n * 512, 512)], ps[:])
                    return go

                def vproj_group(t):
                    def go():
                        ps = spsum.tile([128, 512], f32, tag="s",
                                        name=f"vps{t}")
                        for k in range(NK):
                            nc.tensor.matmul(
                                ps[:], xT_t[:, k, ts(t, 128)], wv_t[:, k],
                                start=(k == 0), stop=(k == NK - 1))
                        nc.vector.tensor_copy(
                            va4[:, t, :, 0:D],
                            ps.rearrange("p (h d) -> p h d", d=D))
                    return go

                def proj_group(t):
                    def go():
                        o_t = opool.tile([128, 1024], f32, tag="o",
                                         name=f"o{t}")
                        ps = spsum.tile([128, 1024], f32, tag="s",
                                        name=f"pps{t}")
                        for n2 in range(2):
                            for kp in range(4):
                                nc.tensor.matmul(
                                    ps[:, ds(n2 * 512, 512)],
                                    yT[:, kp, ts(t, 128)],
                                    wp_t[:, kp, ds(n2 * 512, 512)],
                                    start=(kp == 0), stop=(kp == 3))
                        nc.vector.tensor_copy(o_t[:], ps[:])
                        nc.sync.dma_start(out_d[ds(t * 128, 128)], o_t[:])
                    return go

                deferred_norm = []

                def make_norm(avs, head, p, c):
                    def go():
                        dn = npool.tile([1, 512], f32, tag="dn",
                                        name=f"dn{p}_{c}_{head}")
                        nc.vector.tensor_copy(dn[:], avs[D:D + 1, :])
                        rb = npool.tile([D, 512], f32, tag="rb",
                                        name=f"rb{p}_{c}_{head}")
                        nc.gpsimd.partition_broadcast(rb[:], dn[:])
                        rr = npool.tile([D, 512], f32, tag="rr",
                                        name=f"rr{p}_{c}_{head}")
                        nc.vector.reciprocal_approx_fast(out=rr[:], in_=rb[:])
                        nc.vector.tensor_mul(
                            yT[ds(D * head, D), p, ds(c * 512, 512)],
                            avs[0:D, :], rr[:])
                    return go

                # ---- attention: chunk-major with PE backlog interleave ----
                for c in range(NCH):
                    backlog = []
                    if c + 1 < NCH:
                        for t in range(4 * (c + 1), 4 * (c + 2)):
                            backlog.append(vproj_group(t))
                        for g in range(8):
                            backlog.append(qkproj_group(g // 2, g % 2, c + 1))
                    # output projection for chunk X interleaves two chunks
                    # later (X+2), where its yT norms are long complete; the
                    # ACT-saturated final chunk gets proj(c1) ungated plus
                    # proj(c2) in a second queue paced only after the
                    # (p=0, s==1) norm drain.
                    backlog2 = []
                    if c == 2:
                        for t in range(0, 4):
                            backlog.append(proj_group(t))
                    if c == 3:
                        for t in range(4, 8):
                            backlog.append(proj_group(t))
                        for t in range(8, 12):
                            backlog2.append(proj_group(t))
                    nblk = 4 * (c + 1)
                    nsuper = nblk // 2
                    total_slots = 4 * (nsuper + 2)
                    done_slots = 0
                    emitted = 0
                    emitted2 = 0
                    for p in range(NPAIR):
                        av_A = avpsum.tile([128, 512], f32, tag="av",
                                           name=f"avA{p}_{c}")
                        av_B = avpsum.tile([128, 512], f32, tag="av",
                                           name=f"avB{p}_{c}")
                        pend = {}
                        for s in range(nsuper + 2):
                            if s == 1 and deferred_norm:
                                for fn in deferred_norm:
                                    fn()
                                deferred_norm.clear()
                            if s < nsuper:
                                sA = spsum.tile([128, 1024], f32, tag="s",
                                                name=f"sA{p}_{c}_{s}")
                                sB = spsum.tile([128, 1024], f32, tag="s",
                                                name=f"sB{p}_{c}_{s}")
                                i = s - (nsuper - 2)
                                if i < 0:
                                    for half in (0, 1):
                                        tj = 2 * s + half
                                        nc.tensor.matmul(
                                            sA[:, ds(half * 512, 512)],
                                            qk[0:D, p, 1, ts(tj, 128)],
                                            qk[0:D, p, 0, ds(c * 512, 512)],
                                            start=True, stop=True)
                                        nc.tensor.matmul(
                                            sB[:, ds(half * 512, 512)],
                                            qk[D:128, p, 1, ts(tj, 128)],
                                            qk[D:128, p, 0, ds(c * 512, 512)],
                                            start=True, stop=True)
                                else:
                                    # band: columns [0, 128d) are fully masked
                                    # — compute only the live range
                                    for half in (0, 1):
                                        tj = 2 * s + half
                                        off = 128 * (2 * i + half)
                                        w = 512 - off
                                        nc.tensor.matmul(
                                            sA[:, ds(half * 512 + off, w)],
                                            qk[0:D, p, 1, ts(tj, 128)],
                                            qk[0:D, p, 0, ds(c * 512 + off, w)],
                                            start=True, stop=True)
                                        nc.tensor.matmul(
                                            sB[:, ds(half * 512 + off, w)],
                                            qk[D:128, p, 1, ts(tj, 128)],
                                            qk[D:128, p, 0, ds(c * 512 + off, w)],
                                            start=True, stop=True)
                                e_A = epool.tile([128, 1024], bf16, tag="e",
                                                 name=f"eA{p}_{c}_{s}")
                                e_B = epool.tile([128, 1024], bf16, tag="e",
                                                 name=f"eB{p}_{c}_{s}")
                                if i >= 0:
                                    for half in (0, 1):
                                        off = 128 * (2 * i + half)
                                        w = 512 - off
                                        sl = ds(half * 512 + off, w)
                                        nc.scalar.activation(e_A[:, sl], sA[:, sl],
                                                             Exp, scale=INV_SCALE)
                                        nc.scalar.activation(e_B[:, sl], sB[:, sl],
                                                             Exp, scale=INV_SCALE)
                                        # only the diagonal 128 columns of the
                                        # live range contain masked entries;
                                        # zero them with an in-place triangle
                                        # multiply.
                                        sd = ds(half * 512 + off, 128)
                                        nc.vector.tensor_mul(
                                            e_A[:, sd], e_A[:, sd], mask_t[:, 0:128])
                                        nc.vector.tensor_mul(
                                            e_B[:, sd], e_B[:, sd], mask_t[:, 0:128])
                                else:
                                    nc.scalar.activation(e_A[:], sA[:], Exp,
                                                         scale=INV_SCALE)
                                    nc.scalar.activation(e_B[:], sB[:], Exp,
                                                         scale=INV_SCALE)
                                pend[s] = (e_A, e_B)
                            if s >= 2:
                                e_A, e_B = pend.pop(s - 2)
                                s2 = s - 2
                                for half in (0, 1):
                                    tj = 2 * s2 + half
                                    dd = tj - (nblk - 4)
                                    # masked band columns [0,128d) of E are
                                    # zero after the mask multiply — skip them
                                    off = 128 * dd if dd > 0 else 0
                                    w = 512 - off
                                    nc.tensor.matmul(
                                        av_A[:, ds(off, w)],
                                        v_aug[:, tj, ds(2 * p * 128, 128)],
                                        e_A[:, ds(half * 512 + off, w)],
                                        start=(tj == 0), stop=(tj == nblk - 1))
                                    nc.tensor.matmul(
                                        av_B[:, ds(off, w)],
                                        v_aug[:, tj, ds((2 * p + 1) * 128, 128)],
                                        e_B[:, ds(half * 512 + off, w)],
                                        start=(tj == 0), stop=(tj == nblk - 1))
                            # pace the backlog across the chunk's superslots
                            done_slots += 1
                            want = -(-len(backlog) * done_slots // total_slots)
                            while emitted < want and emitted < len(backlog):
                                backlog[emitted]()
                                emitted += 1
                            if done_slots >= 2 and backlog2:
                                want2 = -(-len(backlog2) * (done_slots - 1)
                                          // (total_slots - 1))
                                while emitted2 < want2 and emitted2 < len(backlog2):
                                    backlog2[emitted2]()
                                    emitted2 += 1
                        # stage av to SBUF with one copy so the PSUM banks free
                        # early; normalize from the staged copy.
                        for head, av in ((0, av_A), (1, av_B)):
                            avs = npool.tile([D + 1, 512], f32, tag="avs",
                                             bufs=4, name=f"avs{p}_{c}_{head}")
                            nc.vector.tensor_copy(avs[:], av[0:D + 1, :])
                            deferred_norm.append(make_norm(avs, head, p, c))

                for fn in deferred_norm:
                    fn()
                deferred_norm.clear()

                # ---- projection tail (last query chunk) ----
                for t in range(12, NT):
                    proj_group(t)()

    nc.compile()
    return nc


def _make_mask():
    # mask[p, j] = 1 iff j >= p: causal triangle in the first 128 cols of a
    # band live range, ones beyond.
    p = np.arange(128)[:, None]
    j = np.arange(512)[None, :]
    return (j >= p).astype(ml_dtypes.bfloat16)


def kernel(x: np.ndarray, W_attn: np.ndarray, W_proj: np.ndarray) -> np.ndarray:
    global LAST_RESULTS
    x = np.asarray(x, dtype=np.float32)
    W_attn = np.asarray(W_attn, dtype=np.float32)
    W_proj = np.asarray(W_proj, dtype=np.float32)

    nc = _cache.get("nc")
    if nc is None:
        nc = _build()
        _cache["nc"] = nc

    mask = _make_mask()
    xTs = [np.ascontiguousarray(x[b].T).astype(ml_dtypes.bfloat16) for b in range(B)]
    in_maps = []
    for cid in range(NCORES):
        b, hh = cid // 2, cid % 2
        qcols = W_attn[:, hh * 512:(hh + 1) * 512]
        kcols = W_attn[:, C + hh * 512:C + (hh + 1) * 512]
        wqk = np.concatenate([qcols, kcols], axis=1)                  # [1024, 1024]
        # pack to [p, ko, pair, m2, mm] -> [128, NK, 1024]
        wqk_pack = np.ascontiguousarray(
            wqk.reshape(NK, 128, 2, NPAIR, 128).transpose(1, 0, 3, 2, 4)
            .reshape(128, NK, NPAIR * 256)
        ).astype(ml_dtypes.bfloat16)
        wv = np.ascontiguousarray(
            W_attn[:, 2 * C + hh * 512:2 * C + (hh + 1) * 512]
        ).astype(ml_dtypes.bfloat16)
        wp = np.ascontiguousarray(W_proj[hh * 512:(hh + 1) * 512, :]).astype(np.float16)
        in_maps.append({
            "xT": xTs[b], "wqk": wqk_pack, "wv": wv, "wp": wp, "mask": mask,
        })

    res = run_bass_kernel_spmd(nc, in_maps, core_ids=list(range(NCORES)))
    LAST_RESULTS = res
    parts = [res.results[cid]["out"] for cid in range(NCORES)]
    out = np.stack([parts[2 * b] + parts[2 * b + 1] for b in range(B)], axis=0)
    return np.ascontiguousarray(out, dtype=np.float32)


# revision 10
# speedup vs baseline: 2.1161x; 1.0014x over previous
"""Batched causal self-attention (B=4, T=2048, C=1024, H=16) on 8 trn2 NeuronCores.

Sharding: data-parallel over B (4) x tensor-parallel over head-halves (2).
Core c handles batch b=c//2, heads [hh*8, hh*8+8) with hh=c%2. Each core
computes its qkv projection slice, causal attention for its 8 heads, and a
partial output projection (512 rows of W_proj); the host sums the two
partials per batch (the TP all-reduce).

v2: chunk-major schedule. The kernel is PE-streaming-bound (~225us of
matmul columns at 2.4GHz); everything else is arranged to keep the PE
continuously fed:
  - startup: k-outer qkv projection waves paced by the xT tile DMAs, so the
    PE starts ~4us in instead of waiting for the full 4MB activation load.
  - chunk-major attention (for c: for pair:) so the output projection for
    chunk c-1 is ready to interleave during chunk c instead of piling up
    behind the last pair (pair-major left an 8us serialized tail).
  - superslots of two key blocks: S^T via two K=64 matmuls per head pair
    (PE row groups), exp on ACT (scale=1/8 folded, no max-subtraction:
    scores ~N(0,0.4^2)), causal mask as an in-place [128,128] multiply on
    only the diagonal block (the rest of the band needs no mask), AV psum
    accumulation with v_aug (64 v-dims + ones column for the denominator +
    zero pad to 128 for FWL) running 2 superslots behind S.
  - independent PE work (next chunk's qkv projection slices, v tiles, and
    chunk c-1's output projection) is emitted into the superslot stream via
    a paced backlog to cover the exp shadow.
  - DMA triggers cost ~0.7us each on the sync queue, so inputs are
    consolidated (wqk packed host-side into one tensor, 2 triggers) and
    outputs merged to one [128,1024] DMA per row tile.
"""

import numpy as np
import ml_dtypes

import concourse.bass as bass
import concourse.mybir as mybir
import concourse.tile as tile
from concourse import bacc
from concourse.bass import ds, ts
from concourse.bass_utils import run_bass_kernel_spmd

B, T, C, H = 4, 2048, 1024, 16
D = 64
NCORES = 8
NPAIR = 4              # head pairs per core (8 heads)
NK = C // 128          # 8 contraction tiles over C
NT = T // 128          # 16 tiles over T
NCH = T // 512         # 4 query chunks
INV_SCALE = 0.125      # 1 / sqrt(C // H)

f32 = mybir.dt.float32
bf16 = mybir.dt.bfloat16
fp16 = mybir.dt.float16

_cache = {}
LAST_RESULTS = None    # test harness reads exec_time_ns from here

# If the caller sets BASS_TRACE=1, run_bass_kernel_spmd imports
# antenv.axon_hooks, which some container images don't ship. Provide a stub
# so tracing degrades gracefully instead of raising ImportError.
try:
    import antenv.axon_hooks  # noqa: F401
except ImportError:
    import sys as _sys
    import types as _types

    _m = _types.ModuleType("antenv.axon_hooks")
    _m._hook = None
    _m.set_axon_ntff_profile_hook = lambda h: setattr(_m, "_hook", h)
    _m.get_axon_ntff_profile_hook = lambda: _m._hook
    _sys.modules["antenv.axon_hooks"] = _m


def _build():
    nc = bacc.Bacc("TRN2", target_bir_lowering=False, debug=False)
    xT_d = nc.dram_tensor("xT", [C, T], bf16, kind="ExternalInput").ap()
    # host-packed: [p, ko, pair*2+m2 flattened to 1024 cols]
    wqk_d = nc.dram_tensor("wqk", [128, NK, NPAIR * 256], bf16,
                           kind="ExternalInput").ap()
    wv_d = nc.dram_tensor("wv", [C, 512], bf16, kind="ExternalInput").ap()
    wp_d = nc.dram_tensor("wp", [512, C], fp16, kind="ExternalInput").ap()
    mask_d = nc.dram_tensor("mask", [128, 512], bf16, kind="ExternalInput").ap()
    out_d = nc.dram_tensor("out", [T, C], f32, kind="ExternalOutput").ap()

    Exp = mybir.ActivationFunctionType.Exp

    with tile.TileContext(nc) as tc:
        with tc.tile_pool(name="persist", bufs=1) as persist:
            xT_t = persist.tile([128, NK, T], bf16, tag="xT")
            wqk_t = persist.tile([128, NK, NPAIR * 256], bf16, tag="wqk")
            wv_t = persist.tile([128, NK, 512], bf16, tag="wv")
            wp_t = persist.tile([128, 4, C], fp16, tag="wp")
            mask_t = persist.tile([128, 512], bf16, tag="mask")
            # per-head blocks padded to 128 cols (v[0:64] | ones at 64 | zeros)
            # so the AV matmul's weight load is exactly 128 columns -> FWL.
            v_aug = persist.tile([128, NT, 8 * 128], bf16, tag="vaug")
            qk = persist.tile([128, NPAIR, 2, T], bf16, tag="qk")
            yT = persist.tile([128, NPAIR, T], fp16, tag="yT")

            # ---- input DMAs, ordered by first use. The startup waves only
            # ---- stream query-chunk-0 columns of xT, so xT is split: cols
            # ---- [0,512) per k-tile early (paces wave A), the rest after wv.
            nc.sync.dma_start(wqk_t[:, 0:1], wqk_d[:, 0:1])
            nc.sync.dma_start(xT_t[:, 0, 0:512], xT_d[ds(0, 128), 0:512])
            nc.sync.dma_start(wqk_t[:, 1:2], wqk_d[:, 1:2])
            nc.sync.dma_start(xT_t[:, 1, 0:512], xT_d[ds(128, 128), 0:512])
            for i in range(1, 4):
                nc.sync.dma_start(wqk_t[:, 2 * i:2 * i + 2],
                                  wqk_d[:, 2 * i:2 * i + 2])
                for k in (2 * i, 2 * i + 1):
                    nc.sync.dma_start(xT_t[:, k, 0:512],
                                      xT_d[ds(k * 128, 128), 0:512])
            nc.sync.dma_start(
                wv_t[:], wv_d.rearrange("(k p) m -> p k m", p=128))
            for k in range(NK):
                nc.sync.dma_start(xT_t[:, k, 512:T],
                                  xT_d[ds(k * 128, 128), 512:T])
            nc.sync.dma_start(mask_t[:], mask_d)
            nc.sync.dma_start(
                wp_t[:], wp_d.rearrange("(kp p) m -> p kp m", p=128))

            # v_aug pad init on DVE (idle during the DMA ramp): zeros in
            # cols [D,128) of each head block, ones column at D.
            va4 = v_aug.rearrange("p n (h e) -> p n h e", e=128)
            nc.vector.memset(va4[:, :, :, D:128].bitcast(mybir.dt.uint16), 0)
            nc.vector.memset(
                va4[:, :, :, D:D + 1].bitcast(mybir.dt.uint16), 0x3F80)

            # ---- startup waves: qk projection for chunk 0 (k-outer, DMA
            # ---- paced), then v tiles 0..3.
            with tc.tile_pool(name="spool", bufs=8, space="PSUM") as spool:
                wave = [
                    spool.tile([128, 512], f32, tag="w", name=f"waveA{g}")
                    for g in range(8)
                ]
                for k in range(NK):
                    for g in range(8):
                        p, m2 = g // 2, g % 2
                        nc.tensor.matmul(
                            wave[g][:],
                            wqk_t[:, k, ds(p * 256 + m2 * 128, 128)],
                            xT_t[:, k, 0:512],
                            start=(k == 0), stop=(k == NK - 1))
                for g in range(8):
                    p, m2 = g // 2, g % 2
                    nc.scalar.copy(qk[:, p, m2, 0:512], wave[g][:])
                vwave = [
                    spool.tile([128, 512], f32, tag="w", name=f"waveV{t}")
                    for t in range(4)
                ]
                for k in range(NK):
                    for t in range(4):
                        nc.tensor.matmul(
                            vwave[t][:], xT_t[:, k, ts(t, 128)], wv_t[:, k],
                            start=(k == 0), stop=(k == NK - 1))
                for t in range(4):
                    nc.vector.tensor_copy(
                        va4[:, t, :, 0:D],
                        vwave[t].rearrange("p (h d) -> p h d", d=D))

            with (
                tc.tile_pool(name="epool", bufs=6) as epool,
                tc.tile_pool(name="npool", bufs=2) as npool,
                tc.tile_pool(name="opool", bufs=2) as opool,
                tc.tile_pool(name="spsum", bufs=3, space="PSUM") as spsum,
                tc.tile_pool(name="avpsum", bufs=2, space="PSUM") as avpsum,
            ):
                # ---- interleavable PE work units ----
                def qkproj_group(p, m2, n):
                    def go():
                        ps = spsum.tile([128, 512], f32, tag="s",
                                        name=f"qkps{p}_{m2}_{n}")
                        for k in range(NK):
                            nc.tensor.matmul(
                                ps[:], wqk_t[:, k, ds(p * 256 + m2 * 128, 128)],
                                xT_t[:, k, ds(n * 512, 512)],
                                start=(k == 0), stop=(k == NK - 1))
                        nc.vector.tensor_copy(
                            qk[:, p, m2, ds(n * 512, 512)], ps[:])
                    return go

                def vproj_group(t):
                    def go():
                        ps = spsum.tile([128, 512], f32, tag="s",
                                        name=f"vps{t}")
                        for k in range(NK):
                            nc.tensor.matmul(
                                ps[:], xT_t[:, k, ts(t, 128)], wv_t[:, k],
                                start=(k == 0), stop=(k == NK - 1))
                        nc.vector.tensor_copy(
                            va4[:, t, :, 0:D],
                            ps.rearrange("p (h d) -> p h d", d=D))
                    return go

                def proj_group(t):
                    def go():
                        o_t = opool.tile([128, 1024], f32, tag="o",
                                         name=f"o{t}")
                        ps = spsum.tile([128, 1024], f32, tag="s",
                                        name=f"pps{t}")
                        for n2 in range(2):
                            for kp in range(4):
                                nc.tensor.matmul(
                                    ps[:, ds(n2 * 512, 512)],
                                    yT[:, kp, ts(t, 128)],
                                    wp_t[:, kp, ds(n2 * 512, 512)],
                                    start=(kp == 0), stop=(kp == 3))
                        # halves pipeline: DMA of half 0 overlaps copy of half 1
                        for n2 in range(2):
                            sl = ds(n2 * 512, 512)
                            nc.vector.tensor_copy(o_t[:, sl], ps[:, sl])
                            nc.sync.dma_start(
                                out_d[ds(t * 128, 128), sl], o_t[:, sl])
                    return go

                deferred_norm = []

                def make_norm(avs, head, p, c):
                    def go():
                        dn = npool.tile([1, 512], f32, tag="dn",
                                        name=f"dn{p}_{c}_{head}")
                        nc.vector.tensor_copy(dn[:], avs[D:D + 1, :])
                        rb = npool.tile([D, 512], f32, tag="rb",
                                        name=f"rb{p}_{c}_{head}")
                        nc.gpsimd.partition_broadcast(rb[:], dn[:])
                        rr = npool.tile([D, 512], f32, tag="rr",
                                        name=f"rr{p}_{c}_{head}")
                        nc.vector.reciprocal_approx_fast(out=rr[:], in_=rb[:])
                        nc.vector.tensor_mul(
                            yT[ds(D * head, D), p, ds(c * 512, 512)],
                            avs[0:D, :], rr[:])
                    return go

                # ---- attention: chunk-major with PE backlog interleave ----
                for c in range(NCH):
                    backlog = []
                    if c + 1 < NCH:
                        for t in range(4 * (c + 1), 4 * (c + 2)):
                            backlog.append(vproj_group(t))
                        for g in range(8):
                            backlog.append(qkproj_group(g // 2, g % 2, c + 1))
                    # output projection for chunk X interleaves two chunks
                    # later (X+2), where its yT norms are long complete; the
                    # ACT-saturated final chunk gets proj(c1) ungated plus
                    # proj(c2) in a second queue paced only after the
                    # (p=0, s==1) norm drain.
                    backlog2 = []
                    if c == 2:
                        for t in range(0, 4):
                            backlog.append(proj_group(t))
                    if c == 3:
                        for t in range(4, 8):
                            backlog.append(proj_group(t))
                        for t in range(8, 12):
                            backlog2.append(proj_group(t))
                    nblk = 4 * (c + 1)
                    nsuper = nblk // 2
                    total_slots = 4 * (nsuper + 2)
                    done_slots = 0
                    emitted = 0
                    emitted2 = 0
                    for p in range(NPAIR):
                        av_A = avpsum.tile([128, 512], f32, tag="av",
                                           name=f"avA{p}_{c}")
                        av_B = avpsum.tile([128, 512], f32, tag="av",
                                           name=f"avB{p}_{c}")
                        pend = {}
                        for s in range(nsuper + 2):
                            if s == 1 and deferred_norm:
                                for fn in deferred_norm:
                                    fn()
                                deferred_norm.clear()
                            if s < nsuper:
                                sA = spsum.tile([128, 1024], f32, tag="s",
                                                name=f"sA{p}_{c}_{s}")
                                sB = spsum.tile([128, 1024], f32, tag="s",
                                                name=f"sB{p}_{c}_{s}")
                                i = s - (nsuper - 2)
                                if i < 0:
                                    for half in (0, 1):
                                        tj = 2 * s + half
                                        nc.tensor.matmul(
                                            sA[:, ds(half * 512, 512)],
                                            qk[0:D, p, 1, ts(tj, 128)],
                                            qk[0:D, p, 0, ds(c * 512, 512)],
                                            start=True, stop=True)
                                        nc.tensor.matmul(
                                            sB[:, ds(half * 512, 512)],
                                            qk[D:128, p, 1, ts(tj, 128)],
                                            qk[D:128, p, 0, ds(c * 512, 512)],
                                            start=True, stop=True)
                                else:
                                    # band: columns [0, 128d) are fully masked
                                    # — compute only the live range
                                    for half in (0, 1):
                                        tj = 2 * s + half
                                        off = 128 * (2 * i + half)
                                        w = 512 - off
                                        nc.tensor.matmul(
                                            sA[:, ds(half * 512 + off, w)],
                                            qk[0:D, p, 1, ts(tj, 128)],
                                            qk[0:D, p, 0, ds(c * 512 + off, w)],
                                            start=True, stop=True)
                                        nc.tensor.matmul(
                                            sB[:, ds(half * 512 + off, w)],
                                            qk[D:128, p, 1, ts(tj, 128)],
                                            qk[D:128, p, 0, ds(c * 512 + off, w)],
                                            start=True, stop=True)
                                e_A = epool.tile([128, 1024], bf16, tag="e",
                                                 name=f"eA{p}_{c}_{s}")
                                e_B = epool.tile([128, 1024], bf16, tag="e",
                                                 name=f"eB{p}_{c}_{s}")
                                if i >= 0:
                                    for half in (0, 1):
                                        off = 128 * (2 * i + half)
                                        w = 512 - off
                                        sl = ds(half * 512 + off, w)
                                        nc.scalar.activation(e_A[:, sl], sA[:, sl],
                                                             Exp, scale=INV_SCALE)
                                        nc.scalar.activation(e_B[:, sl], sB[:, sl],
                                                             Exp, scale=INV_SCALE)
                                        # only the diagonal 128 columns of the
                                        # live range contain masked entries;
                                        # zero them with an in-place triangle
                                        # multiply.
                                        sd = ds(half * 512 + off, 128)
                                        nc.vector.tensor_mul(
                                            e_A[:, sd], e_A[:, sd], mask_t[:, 0:128])
                                        nc.vector.tensor_mul(
                                            e_B[:, sd], e_B[:, sd], mask_t[:, 0:128])
                                else:
                                    nc.scalar.activation(e_A[:], sA[:], Exp,
                                                         scale=INV_SCALE)
                                    nc.scalar.activation(e_B[:], sB[:], Exp,
                                                         scale=INV_SCALE)
                                pend[s] = (e_A, e_B)
                            if s >= 2:
                                e_A, e_B = pend.pop(s - 2)
                                s2 = s - 2
                                for half in (0, 1):
                                    tj = 2 * s2 + half
                                    dd = tj - (nblk - 4)
                                    # masked band columns [0,128d) of E are
                                    # zero after the mask multiply — skip them
                                    off = 128 * dd if dd > 0 else 0
                                    w = 512 - off
                                    nc.tensor.matmul(
                                        av_A[:, ds(off, w)],
                                        v_aug[:, tj, ds(2 * p * 128, 128)],
                                        e_A[:, ds(half * 512 + off, w)],
                                        start=(tj == 0), stop=(tj == nblk - 1))
                                    nc.tensor.matmul(
                                        av_B[:, ds(off, w)],
                                        v_aug[:, tj, ds((2 * p + 1) * 128, 128)],
                                        e_B[:, ds(half * 512 + off, w)],
                                        start=(tj == 0), stop=(tj == nblk - 1))
                            # pace the backlog across the chunk's superslots
                            done_slots += 1
                            want = -(-len(backlog) * done_slots // total_slots)
                            while emitted < want and emitted < len(backlog):
                                backlog[emitted]()
                                emitted += 1
                            if done_slots >= 2 and backlog2:
                                half = max(1, (total_slots - 1) // 2)
                                want2 = -(-len(backlog2) * (done_slots - 1)
                                          // half)
                                while emitted2 < want2 and emitted2 < len(backlog2):
                                    backlog2[emitted2]()
                                    emitted2 += 1
                        # stage av to SBUF with one copy so the PSUM banks free
                        # early; normalize from the staged copy.
                        for head, av in ((0, av_A), (1, av_B)):
                            avs = npool.tile([D + 1, 512], f32, tag="avs",
                                             bufs=4, name=f"avs{p}_{c}_{head}")
                            nc.vector.tensor_copy(avs[:], av[0:D + 1, :])
                            deferred_norm.append(make_norm(avs, head, p, c))

                for fn in deferred_norm:
                    fn()
                deferred_norm.clear()

                # ---- projection tail (last query chunk) ----
                for t in range(12, NT):
                    proj_group(t)()

    nc.compile()
    return nc


def _make_mask():
    # mask[p, j] = 1 iff j >= p: causal triangle in the first 128 cols of a
    # band live range, ones beyond.
    p = np.arange(128)[:, None]
    j = np.arange(512)[None, :]
    return (j >= p).astype(ml_dtypes.bfloat16)


def kernel(x: np.ndarray, W_attn: np.ndarray, W_proj: np.ndarray) -> np.ndarray:
    global LAST_RESULTS
    x = np.asarray(x, dtype=np.float32)
    W_attn = np.asarray(W_attn, dtype=np.float32)
    W_proj = np.asarray(W_proj, dtype=np.float32)

    nc = _cache.get("nc")
    if nc is None:
        nc = _build()
        _cache["nc"] = nc

    mask = _make_mask()
    xTs = [np.ascontiguousarray(x[b].T).astype(ml_dtypes.bfloat16) for b in range(B)]
    in_maps = []
    for cid in range(NCORES):
        b, hh = cid // 2, cid % 2
        qcols = W_attn[:, hh * 512:(hh + 1) * 512]
        kcols = W_attn[:, C + hh * 512:C + (hh + 1) * 512]
        wqk = np.concatenate([qcols, kcols], axis=1)                  # [1024, 1024]
        # pack to [p, ko, pair, m2, mm] -> [128, NK, 1024]
        wqk_pack = np.ascontiguousarray(
            wqk.reshape(NK, 128, 2, NPAIR, 128).transpose(1, 0, 3, 2, 4)
            .reshape(128, NK, NPAIR * 256)
        ).astype(ml_dtypes.bfloat16)
        wv = np.ascontiguousarray(
            W_attn[:, 2 * C + hh * 512:2 * C + (hh + 1) * 512]
        ).astype(ml_dtypes.bfloat16)
        wp = np.ascontiguousarray(W_proj[hh * 512:(hh + 1) * 512, :]).astype(np.float16)
        in_maps.append({
            "xT": xTs[b], "wqk": wqk_pack, "wv": wv, "wp": wp, "mask": mask,
        })

    res = run_bass_kernel_spmd(nc, in_maps, core_ids=list(range(NCORES)))
    LAST_RESULTS = res
    parts = [res.results[cid]["out"] for cid in range(NCORES)]
    out = np.stack([parts[2 * b] + parts[2 * b + 1] for b in range(B)], axis=0)
    return np.ascontiguousarray(out, dtype=np.float32)


# revision 15
# speedup vs baseline: 2.1354x; 1.0091x over previous
"""Batched causal self-attention (B=4, T=2048, C=1024, H=16) on 8 trn2 NeuronCores.

Sharding: data-parallel over B (4) x tensor-parallel over head-halves (2).
Core c handles batch b=c//2, heads [hh*8, hh*8+8) with hh=c%2. Each core
computes its qkv projection slice, causal attention for its 8 heads, and a
partial output projection (512 rows of W_proj); the host sums the two
partials per batch (the TP all-reduce).

v2: chunk-major schedule. The kernel is PE-streaming-bound (~225us of
matmul columns at 2.4GHz); everything else is arranged to keep the PE
continuously fed:
  - startup: k-outer qkv projection waves paced by the xT tile DMAs, so the
    PE starts ~4us in instead of waiting for the full 4MB activation load.
  - chunk-major attention (for c: for pair:) so the output projection for
    chunk c-1 is ready to interleave during chunk c instead of piling up
    behind the last pair (pair-major left an 8us serialized tail).
  - superslots of two key blocks: S^T via two K=64 matmuls per head pair
    (PE row groups), exp on ACT (scale=1/8 folded, no max-subtraction:
    scores ~N(0,0.4^2)), causal mask as an in-place [128,128] multiply on
    only the diagonal block (the rest of the band needs no mask), AV psum
    accumulation with v_aug (64 v-dims + ones column for the denominator +
    zero pad to 128 for FWL) running 2 superslots behind S.
  - independent PE work (next chunk's qkv projection slices, v tiles, and
    chunk c-1's output projection) is emitted into the superslot stream via
    a paced backlog to cover the exp shadow.
  - DMA triggers cost ~0.7us each on the sync queue, so inputs are
    consolidated (wqk packed host-side into one tensor, 2 triggers) and
    outputs merged to one [128,1024] DMA per row tile.
"""

import numpy as np
import ml_dtypes

import concourse.bass as bass
import concourse.mybir as mybir
import concourse.tile as tile
from concourse import bacc
from concourse.bass import ds, ts
from concourse.bass_utils import run_bass_kernel_spmd

B, T, C, H = 4, 2048, 1024, 16
D = 64
NCORES = 8
NPAIR = 4              # head pairs per core (8 heads)
NK = C // 128          # 8 contraction tiles over C
NT = T // 128          # 16 tiles over T
NCH = T // 512         # 4 query chunks
INV_SCALE = 0.125      # 1 / sqrt(C // H)

f32 = mybir.dt.float32
bf16 = mybir.dt.bfloat16
fp16 = mybir.dt.float16

_cache = {}
LAST_RESULTS = None    # test harness reads exec_time_ns from here

# If the caller sets BASS_TRACE=1, run_bass_kernel_spmd imports
# antenv.axon_hooks, which some container images don't ship. Provide a stub
# so tracing degrades gracefully instead of raising ImportError.
try:
    import antenv.axon_hooks  # noqa: F401
except ImportError:
    import sys as _sys
    import types as _types

    _m = _types.ModuleType("antenv.axon_hooks")
    _m._hook = None
    _m.set_axon_ntff_profile_hook = lambda h: setattr(_m, "_hook", h)
    _m.get_axon_ntff_profile_hook = lambda: _m._hook
    _sys.modules["antenv.axon_hooks"] = _m


def _build():
    nc = bacc.Bacc("TRN2", target_bir_lowering=False, debug=False)
    xT_d = nc.dram_tensor("xT", [C, T], bf16, kind="ExternalInput").ap()
    # host-packed: [p, ko, pair*2+m2 flattened to 1024 cols]
    wqk_d = nc.dram_tensor("wqk", [128, NK, NPAIR * 256], bf16,
                           kind="ExternalInput").ap()
    wv_d = nc.dram_tensor("wv", [C, 512], bf16, kind="ExternalInput").ap()
    wp_d = nc.dram_tensor("wp", [512, C], fp16, kind="ExternalInput").ap()
    mask_d = nc.dram_tensor("mask", [128, 512], bf16, kind="ExternalInput").ap()
    out_d = nc.dram_tensor("out", [T, C], fp16, kind="ExternalOutput").ap()

    Exp = mybir.ActivationFunctionType.Exp

    with tile.TileContext(nc) as tc:
        with tc.tile_pool(name="persist", bufs=1) as persist:
            xT_t = persist.tile([128, NK, T], bf16, tag="xT")
            wqk_t = persist.tile([128, NK, NPAIR * 256], bf16, tag="wqk")
            wv_t = persist.tile([128, NK, 512], bf16, tag="wv")
            wp_t = persist.tile([128, 4, C], fp16, tag="wp")
            mask_t = persist.tile([128, 512], bf16, tag="mask")
            # per-head blocks padded to 128 cols (v[0:64] | ones at 64 | zeros)
            # so the AV matmul's weight load is exactly 128 columns -> FWL.
            v_aug = persist.tile([128, NT, 8 * 128], bf16, tag="vaug")
            qk = persist.tile([128, NPAIR, 2, T], bf16, tag="qk")
            yT = persist.tile([128, NPAIR, T], fp16, tag="yT")

            # ---- input DMAs, ordered by first use. The startup waves only
            # ---- stream query-chunk-0 columns of xT, so xT is split: cols
            # ---- [0,512) per k-tile early (paces wave A), the rest after wv.
            nc.sync.dma_start(wqk_t[:, 0:1], wqk_d[:, 0:1])
            nc.sync.dma_start(xT_t[:, 0, 0:512], xT_d[ds(0, 128), 0:512])
            nc.sync.dma_start(wqk_t[:, 1:2], wqk_d[:, 1:2])
            nc.sync.dma_start(xT_t[:, 1, 0:512], xT_d[ds(128, 128), 0:512])
            for i in range(1, 4):
                nc.sync.dma_start(wqk_t[:, 2 * i:2 * i + 2],
                                  wqk_d[:, 2 * i:2 * i + 2])
                for k in (2 * i, 2 * i + 1):
                    nc.sync.dma_start(xT_t[:, k, 0:512],
                                      xT_d[ds(k * 128, 128), 0:512])
            nc.sync.dma_start(
                wv_t[:], wv_d.rearrange("(k p) m -> p k m", p=128))
            for k in range(NK):
                nc.sync.dma_start(xT_t[:, k, 512:T],
                                  xT_d[ds(k * 128, 128), 512:T])
            nc.sync.dma_start(mask_t[:], mask_d)
            nc.sync.dma_start(
                wp_t[:], wp_d.rearrange("(kp p) m -> p kp m", p=128))

            # v_aug pad init on DVE (idle during the DMA ramp): zeros in
            # cols [D,128) of each head block, ones column at D.
            va4 = v_aug.rearrange("p n (h e) -> p n h e", e=128)
            nc.vector.memset(va4[:, :, :, D:128].bitcast(mybir.dt.uint16), 0)
            nc.vector.memset(
                va4[:, :, :, D:D + 1].bitcast(mybir.dt.uint16), 0x3F80)
            ones_row = persist.tile([1, D], bf16, tag="ones")
            nc.vector.memset(ones_row.bitcast(mybir.dt.uint16), 0x3F80)

            # ---- startup waves: qk projection for chunk 0 (k-outer, DMA
            # ---- paced), then v tiles 0..3.
            with tc.tile_pool(name="spool", bufs=8, space="PSUM") as spool:
                wave = [
                    spool.tile([128, 512], f32, tag="w", name=f"waveA{g}")
                    for g in range(8)
                ]
                for k in range(NK):
                    for g in range(8):
                        p, m2 = g // 2, g % 2
                        nc.tensor.matmul(
                            wave[g][:],
                            wqk_t[:, k, ds(p * 256 + m2 * 128, 128)],
                            xT_t[:, k, 0:512],
                            start=(k == 0), stop=(k == NK - 1))
                for g in range(8):
                    p, m2 = g // 2, g % 2
                    nc.scalar.copy(qk[:, p, m2, 0:512], wave[g][:])
                for t in range(4):
                    vps = spool.tile([128, 512], f32, tag="w", name=f"waveV{t}")
                    for k in range(NK):
                        nc.tensor.matmul(
                            vps[:], xT_t[:, k, ts(t, 128)], wv_t[:, k],
                            start=(k == 0), stop=(k == NK - 1))
                    nc.vector.tensor_copy(
                        va4[:, t, :, 0:D],
                        vps.rearrange("p (h d) -> p h d", d=D))

            with (
                tc.tile_pool(name="epool", bufs=6) as epool,
                tc.tile_pool(name="npool", bufs=2) as npool,
                tc.tile_pool(name="opool", bufs=2) as opool,
                tc.tile_pool(name="spsum", bufs=3, space="PSUM") as spsum,
                tc.tile_pool(name="avpsum", bufs=2, space="PSUM") as avpsum,
            ):
                # ---- interleavable PE work units ----
                def qkproj_group(p, m2, n):
                    def go():
                        ps = spsum.tile([128, 512], f32, tag="s",
                                        name=f"qkps{p}_{m2}_{n}")
                        for k in range(NK):
                            nc.tensor.matmul(
                                ps[:], wqk_t[:, k, ds(p * 256 + m2 * 128, 128)],
                                xT_t[:, k, ds(n * 512, 512)],
                                start=(k == 0), stop=(k == NK - 1))
                        nc.vector.tensor_copy(
                            qk[:, p, m2, ds(n * 512, 512)], ps[:])
                    return go

                def vproj_group(t):
                    def go():
                        ps = spsum.tile([128, 512], f32, tag="s",
                                        name=f"vps{t}")
                        for k in range(NK):
                            nc.tensor.matmul(
                                ps[:], xT_t[:, k, ts(t, 128)], wv_t[:, k],
                                start=(k == 0), stop=(k == NK - 1))
                        nc.vector.tensor_copy(
                            va4[:, t, :, 0:D],
                            ps.rearrange("p (h d) -> p h d", d=D))
                    return go

                def proj_group(t):
                    def go():
                        o_t = opool.tile([128, 1024], fp16, tag="o",
                                         name=f"o{t}")
                        ps = spsum.tile([128, 1024], f32, tag="s",
                                        name=f"pps{t}")
                        for n2 in range(2):
                            for kp in range(4):
                                nc.tensor.matmul(
                                    ps[:, ds(n2 * 512, 512)],
                                    yT[:, kp, ts(t, 128)],
                                    wp_t[:, kp, ds(n2 * 512, 512)],
                                    start=(kp == 0), stop=(kp == 3))
                        # halves pipeline: DMA of half 0 overlaps copy of half 1
                        for n2 in range(2):
                            sl = ds(n2 * 512, 512)
                            nc.vector.tensor_copy(o_t[:, sl], ps[:, sl])
                            nc.sync.dma_start(
                                out_d[ds(t * 128, 128), sl], o_t[:, sl])
                    return go

                deferred_norm = []

                def make_norm(avs, head, p, c):
                    def go():
                        dn = npool.tile([1, 512], f32, tag="dn",
                                        name=f"dn{p}_{c}_{head}")
                        nc.vector.tensor_copy(dn[:], avs[D:D + 1, :])
                        rb = npool.tile([D, 512], f32, tag="rb",
                                        name=f"rb{p}_{c}_{head}")
                        nc.gpsimd.partition_broadcast(rb[:], dn[:])
                        rr = npool.tile([D, 512], f32, tag="rr",
                                        name=f"rr{p}_{c}_{head}")
                        nc.vector.reciprocal_approx_fast(out=rr[:], in_=rb[:])
                        nc.vector.tensor_mul(
                            yT[ds(D * head, D), p, ds(c * 512, 512)],
                            avs[0:D, :], rr[:])
                    return go

                # ---- attention: chunk-major with PE backlog interleave ----
                for c in range(NCH):
                    backlog = []
                    if c + 1 < NCH:
                        for t in range(4 * (c + 1), 4 * (c + 2)):
                            backlog.append(vproj_group(t))
                        for g in range(8):
                            backlog.append(qkproj_group(g // 2, g % 2, c + 1))
                    # output projection for chunk X interleaves two chunks
                    # later (X+2), where its yT norms are long complete; the
                    # ACT-saturated final chunk gets proj(c1) ungated plus
                    # proj(c2) in a second queue paced only after the
                    # (p=0, s==1) norm drain.
                    backlog2 = []
                    if c == 2:
                        for t in range(0, 4):
                            backlog.append(proj_group(t))
                    if c == 3:
                        for t in range(4, 8):
                            backlog.append(proj_group(t))
                        for t in range(8, 12):
                            backlog2.append(proj_group(t))
                    nblk = 4 * (c + 1)
                    nsuper = nblk // 2
                    total_slots = 4 * (nsuper + 2)
                    done_slots = 0
                    emitted = 0
                    emitted2 = 0
                    for p in range(NPAIR):
                        av_A = avpsum.tile([128, 512], f32, tag="av",
                                           name=f"avA{p}_{c}")
                        av_B = avpsum.tile([128, 512], f32, tag="av",
                                           name=f"avB{p}_{c}")
                        pend = {}
                        for s in range(nsuper + 2):
                            if s == 1 and deferred_norm:
                                for fn in deferred_norm:
                                    fn()
                                deferred_norm.clear()
                            if s < nsuper:
                                sA = spsum.tile([128, 1024], f32, tag="s",
                                                name=f"sA{p}_{c}_{s}")
                                sB = spsum.tile([128, 1024], f32, tag="s",
                                                name=f"sB{p}_{c}_{s}")
                                i = s - (nsuper - 2)
                                if i < 0:
                                    for half in (0, 1):
                                        tj = 2 * s + half
                                        nc.tensor.matmul(
                                            sA[:, ds(half * 512, 512)],
                                            qk[0:D, p, 1, ts(tj, 128)],
                                            qk[0:D, p, 0, ds(c * 512, 512)],
                                            start=True, stop=True)
                                        nc.tensor.matmul(
                                            sB[:, ds(half * 512, 512)],
                                            qk[D:128, p, 1, ts(tj, 128)],
                                            qk[D:128, p, 0, ds(c * 512, 512)],
                                            start=True, stop=True)
                                else:
                                    # band: columns [0, 128d) are fully masked
                                    # — compute only the live range
                                    for half in (0, 1):
                                        tj = 2 * s + half
                                        off = 128 * (2 * i + half)
                                        w = 512 - off
                                        nc.tensor.matmul(
                                            sA[:, ds(half * 512 + off, w)],
                                            qk[0:D, p, 1, ts(tj, 128)],
                                            qk[0:D, p, 0, ds(c * 512 + off, w)],
                                            start=True, stop=True)
                                        nc.tensor.matmul(
                                            sB[:, ds(half * 512 + off, w)],
                                            qk[D:128, p, 1, ts(tj, 128)],
                                            qk[D:128, p, 0, ds(c * 512 + off, w)],
                                            start=True, stop=True)
                                e_A = epool.tile([128, 1024], bf16, tag="e",
                                                 name=f"eA{p}_{c}_{s}")
                                e_B = epool.tile([128, 1024], bf16, tag="e",
                                                 name=f"eB{p}_{c}_{s}")
                                if i >= 0:
                                    for half in (0, 1):
                                        off = 128 * (2 * i + half)
                                        w = 512 - off
                                        sl = ds(half * 512 + off, w)
                                        nc.scalar.activation(e_A[:, sl], sA[:, sl],
                                                             Exp, scale=INV_SCALE)
                                        nc.scalar.activation(e_B[:, sl], sB[:, sl],
                                                             Exp, scale=INV_SCALE)
                                        # only the diagonal 128 columns of the
                                        # live range contain masked entries;
                                        # zero them with an in-place triangle
                                        # multiply.
                                        sd = ds(half * 512 + off, 128)
                                        nc.vector.tensor_mul(
                                            e_A[:, sd], e_A[:, sd], mask_t[:, 0:128])
                                        nc.vector.tensor_mul(
                                            e_B[:, sd], e_B[:, sd], mask_t[:, 0:128])
                                else:
                                    nc.scalar.activation(e_A[:], sA[:], Exp,
                                                         scale=INV_SCALE)
                                    nc.scalar.activation(e_B[:], sB[:], Exp,
                                                         scale=INV_SCALE)
                                pend[s] = (e_A, e_B)
                            if s >= 2:
                                e_A, e_B = pend.pop(s - 2)
                                s2 = s - 2
                                for half in (0, 1):
                                    tj = 2 * s2 + half
                                    dd = tj - (nblk - 4)
                                    # masked band columns [0,128d) of E are
                                    # zero after the mask multiply — skip them
                                    off = 128 * dd if dd > 0 else 0
                                    w = 512 - off
                                    nc.tensor.matmul(
                                        av_A[:, ds(off, w)],
                                        v_aug[:, tj, ds(2 * p * 128, 128)],
                                        e_A[:, ds(half * 512 + off, w)],
                                        start=(tj == 0), stop=(tj == nblk - 1))
                                    nc.tensor.matmul(
                                        av_B[:, ds(off, w)],
                                        v_aug[:, tj, ds((2 * p + 1) * 128, 128)],
                                        e_B[:, ds(half * 512 + off, w)],
                                        start=(tj == 0), stop=(tj == nblk - 1))
                            # pace the backlog across the chunk's superslots
                            done_slots += 1
                            want = -(-len(backlog) * done_slots // total_slots)
                            while emitted < want and emitted < len(backlog):
                                backlog[emitted]()
                                emitted += 1
                            if done_slots >= 2 and backlog2:
                                half = max(1, (total_slots - 1) // 2)
                                want2 = -(-len(backlog2) * (done_slots - 1)
                                          // half)
                                while emitted2 < want2 and emitted2 < len(backlog2):
                                    backlog2[emitted2]()
                                    emitted2 += 1
                        # stage av to SBUF with one copy so the PSUM banks free
                        # early; normalize from the staged copy.
                        for head, av in ((0, av_A), (1, av_B)):
                            avs = npool.tile([D + 1, 512], f32, tag="avs",
                                             bufs=4, name=f"avs{p}_{c}_{head}")
                            nc.vector.tensor_copy(avs[:], av[0:D + 1, :])
                            deferred_norm.append(make_norm(avs, head, p, c))

                for fn in deferred_norm:
                    fn()
                deferred_norm.clear()

                # ---- projection tail (last query chunk) ----
                for t in range(12, NT):
                    proj_group(t)()

    nc.compile()
    return nc


def _make_mask():
    # mask[p, j] = 1 iff j >= p: causal triangle in the first 128 cols of a
    # band live range, ones beyond.
    p = np.arange(128)[:, None]
    j = np.arange(512)[None, :]
    return (j >= p).astype(ml_dtypes.bfloat16)


def kernel(x: np.ndarray, W_attn: np.ndarray, W_proj: np.ndarray) -> np.ndarray:
    global LAST_RESULTS
    x = np.asarray(x, dtype=np.float32)
    W_attn = np.asarray(W_attn, dtype=np.float32)
    W_proj = np.asarray(W_proj, dtype=np.float32)

    nc = _cache.get("nc")
    if nc is None:
        nc = _build()
        _cache["nc"] = nc

    mask = _make_mask()
    xTs = [np.ascontiguousarray(x[b].T).astype(ml_dtypes.bfloat16) for b in range(B)]
    in_maps = []
    for cid in range(NCORES):
        b, hh = cid // 2, cid % 2
        qcols = W_attn[:, hh * 512:(hh + 1) * 512]
        kcols = W_attn[:, C + hh * 512:C + (hh + 1) * 512]
        wqk = np.concatenate([qcols, kcols], axis=1)                  # [1024, 1024]
        # pack to [p, ko, pair, m2, mm] -> [128, NK, 1024]
        wqk_pack = np.ascontiguousarray(
            wqk.reshape(NK, 128, 2, NPAIR, 128).transpose(1, 0, 3, 2, 4)
            .reshape(128, NK, NPAIR * 256)
        ).astype(ml_dtypes.bfloat16)
        wv = np.ascontiguousarray(
            W_attn[:, 2 * C + hh * 512:2 * C + (hh + 1) * 512]
        ).astype(ml_dtypes.bfloat16)
        wp = np.ascontiguousarray(W_proj[hh * 512:(hh + 1) * 512, :]).astype(np.float16)
        in_maps.append({
            "xT": xTs[b], "wqk": wqk_pack, "wv": wv, "wp": wp, "mask": mask,
        })

    res = run_bass_kernel_spmd(nc, in_maps, core_ids=list(range(NCORES)))
    LAST_RESULTS = res
    parts = [res.results[cid]["out"].astype(np.float32) for cid in range(NCORES)]
    out = np.stack([parts[2 * b] + parts[2 * b + 1] for b in range(B)], axis=0)
    return np.ascontiguousarray(out, dtype=np.float32)


# revision 17
# speedup vs baseline: 2.1366x; 1.0006x over previous
"""Batched causal self-attention (B=4, T=2048, C=1024, H=16) on 8 trn2 NeuronCores.

Sharding: data-parallel over B (4) x tensor-parallel over head-halves (2).
Core c handles batch b=c//2, heads [hh*8, hh*8+8) with hh=c%2. Each core
computes its qkv projection slice, causal attention for its 8 heads, and a
partial output projection (512 rows of W_proj); the host sums the two
partials per batch (the TP all-reduce).

v2: chunk-major schedule. The kernel is PE-streaming-bound (~225us of
matmul columns at 2.4GHz); everything else is arranged to keep the PE
continuously fed:
  - startup: k-outer qkv projection waves paced by the xT tile DMAs, so the
    PE starts ~4us in instead of waiting for the full 4MB activation load.
  - chunk-major attention (for c: for pair:) so the output projection for
    chunk c-1 is ready to interleave during chunk c instead of piling up
    behind the last pair (pair-major left an 8us serialized tail).
  - superslots of two key blocks: S^T via two K=64 matmuls per head pair
    (PE row groups), exp on ACT (scale=1/8 folded, no max-subtraction:
    scores ~N(0,0.4^2)), causal mask as an in-place [128,128] multiply on
    only the diagonal block (the rest of the band needs no mask), AV psum
    accumulation with v_aug (64 v-dims + ones column for the denominator +
    zero pad to 128 for FWL) running 2 superslots behind S.
  - independent PE work (next chunk's qkv projection slices, v tiles, and
    chunk c-1's output projection) is emitted into the superslot stream via
    a paced backlog to cover the exp shadow.
  - DMA triggers cost ~0.7us each on the sync queue, so inputs are
    consolidated (wqk packed host-side into one tensor, 2 triggers) and
    outputs merged to one [128,1024] DMA per row tile.
"""

import numpy as np
import ml_dtypes

import concourse.bass as bass
import concourse.mybir as mybir
import concourse.tile as tile
from concourse import bacc
from concourse.bass import ds, ts
from concourse.bass_utils import run_bass_kernel_spmd

B, T, C, H = 4, 2048, 1024, 16
D = 64
NCORES = 8
NPAIR = 4              # head pairs per core (8 heads)
NK = C // 128          # 8 contraction tiles over C
NT = T // 128          # 16 tiles over T
NCH = T // 512         # 4 query chunks
INV_SCALE = 0.125      # 1 / sqrt(C // H)

f32 = mybir.dt.float32
bf16 = mybir.dt.bfloat16
fp16 = mybir.dt.float16

_cache = {}
LAST_RESULTS = None    # test harness reads exec_time_ns from here

# If the caller sets BASS_TRACE=1, run_bass_kernel_spmd imports
# antenv.axon_hooks, which some container images don't ship. Provide a stub
# so tracing degrades gracefully instead of raising ImportError.
try:
    import antenv.axon_hooks  # noqa: F401
except ImportError:
    import sys as _sys
    import types as _types

    _m = _types.ModuleType("antenv.axon_hooks")
    _m._hook = None
    _m.set_axon_ntff_profile_hook = lambda h: setattr(_m, "_hook", h)
    _m.get_axon_ntff_profile_hook = lambda: _m._hook
    _sys.modules["antenv.axon_hooks"] = _m


def _build():
    nc = bacc.Bacc("TRN2", target_bir_lowering=False, debug=False)
    xT_d = nc.dram_tensor("xT", [C, T], bf16, kind="ExternalInput").ap()
    # host-packed: [p, ko, pair*2+m2 flattened to 1024 cols]
    wqk_d = nc.dram_tensor("wqk", [128, NK, NPAIR * 256], bf16,
                           kind="ExternalInput").ap()
    wv_d = nc.dram_tensor("wv", [C, 512], bf16, kind="ExternalInput").ap()
    wp_d = nc.dram_tensor("wp", [512, C], fp16, kind="ExternalInput").ap()
    mask_d = nc.dram_tensor("mask", [128, 512], bf16, kind="ExternalInput").ap()
    out_d = nc.dram_tensor("out", [T, C], fp16, kind="ExternalOutput").ap()

    Exp = mybir.ActivationFunctionType.Exp

    with tile.TileContext(nc) as tc:
        with tc.tile_pool(name="persist", bufs=1) as persist:
            xT_t = persist.tile([128, NK, T], bf16, tag="xT")
            wqk_t = persist.tile([128, NK, NPAIR * 256], bf16, tag="wqk")
            wv_t = persist.tile([128, NK, 512], bf16, tag="wv")
            wp_t = persist.tile([128, 4, C], fp16, tag="wp")
            mask_t = persist.tile([128, 512], bf16, tag="mask")
            # per-head blocks padded to 128 cols (v[0:64] | ones at 64 | zeros)
            # so the AV matmul's weight load is exactly 128 columns -> FWL.
            v_aug = persist.tile([128, NT, 8 * 128], bf16, tag="vaug")
            qk = persist.tile([128, NPAIR, 2, T], bf16, tag="qk")
            yT = persist.tile([128, NPAIR, T], fp16, tag="yT")

            # ---- input DMAs, ordered by first use. The startup waves only
            # ---- stream query-chunk-0 columns of xT, so xT is split: cols
            # ---- [0,512) per k-tile early (paces wave A), the rest after wv.
            nc.sync.dma_start(wqk_t[:, 0:1], wqk_d[:, 0:1])
            nc.sync.dma_start(xT_t[:, 0, 0:512], xT_d[ds(0, 128), 0:512])
            nc.sync.dma_start(wqk_t[:, 1:2], wqk_d[:, 1:2])
            nc.sync.dma_start(xT_t[:, 1, 0:512], xT_d[ds(128, 128), 0:512])
            for i in range(1, 4):
                nc.sync.dma_start(wqk_t[:, 2 * i:2 * i + 2],
                                  wqk_d[:, 2 * i:2 * i + 2])
                for k in (2 * i, 2 * i + 1):
                    nc.sync.dma_start(xT_t[:, k, 0:512],
                                      xT_d[ds(k * 128, 128), 0:512])
            nc.sync.dma_start(
                wv_t[:], wv_d.rearrange("(k p) m -> p k m", p=128))
            for k in range(NK):
                nc.sync.dma_start(xT_t[:, k, 512:T],
                                  xT_d[ds(k * 128, 128), 512:T])
            nc.sync.dma_start(mask_t[:], mask_d)
            nc.sync.dma_start(
                wp_t[:], wp_d.rearrange("(kp p) m -> p kp m", p=128))

            # v_aug pad init on DVE (idle during the DMA ramp): zeros in
            # cols [D,128) of each head block, ones column at D.
            va4 = v_aug.rearrange("p n (h e) -> p n h e", e=128)
            nc.vector.memset(va4[:, :, :, D:128].bitcast(mybir.dt.uint16), 0)
            nc.vector.memset(
                va4[:, :, :, D:D + 1].bitcast(mybir.dt.uint16), 0x3F80)
            ones_row = persist.tile([1, D], bf16, tag="ones")
            nc.vector.memset(ones_row.bitcast(mybir.dt.uint16), 0x3F80)

            # ---- startup waves: qk projection for chunk 0 (k-outer, DMA
            # ---- paced), then v tiles 0..3.
            with tc.tile_pool(name="spool", bufs=8, space="PSUM") as spool:
                wave = [
                    spool.tile([128, 512], f32, tag="w", name=f"waveA{g}")
                    for g in range(8)
                ]
                for k in range(NK):
                    for g in range(8):
                        p, m2 = g // 2, g % 2
                        nc.tensor.matmul(
                            wave[g][:],
                            wqk_t[:, k, ds(p * 256 + m2 * 128, 128)],
                            xT_t[:, k, 0:512],
                            start=(k == 0), stop=(k == NK - 1))
                for g in range(8):
                    p, m2 = g // 2, g % 2
                    nc.scalar.copy(qk[:, p, m2, 0:512], wave[g][:])
                for t in range(4):
                    vps = spool.tile([128, 512], f32, tag="w", name=f"waveV{t}")
                    for k in range(NK):
                        nc.tensor.matmul(
                            vps[:], xT_t[:, k, ts(t, 128)], wv_t[:, k],
                            start=(k == 0), stop=(k == NK - 1))
                    nc.vector.tensor_copy(
                        va4[:, t, :, 0:D],
                        vps.rearrange("p (h d) -> p h d", d=D))

            with (
                tc.tile_pool(name="epool", bufs=8) as epool,
                tc.tile_pool(name="npool", bufs=2) as npool,
                tc.tile_pool(name="opool", bufs=2) as opool,
                tc.tile_pool(name="spsum", bufs=3, space="PSUM") as spsum,
                tc.tile_pool(name="avpsum", bufs=2, space="PSUM") as avpsum,
            ):
                # ---- interleavable PE work units ----
                def qkproj_group(p, m2, n):
                    def go():
                        ps = spsum.tile([128, 512], f32, tag="s",
                                        name=f"qkps{p}_{m2}_{n}")
                        for k in range(NK):
                            nc.tensor.matmul(
                                ps[:], wqk_t[:, k, ds(p * 256 + m2 * 128, 128)],
                                xT_t[:, k, ds(n * 512, 512)],
                                start=(k == 0), stop=(k == NK - 1))
                        nc.vector.tensor_copy(
                            qk[:, p, m2, ds(n * 512, 512)], ps[:])
                    return go

                def vproj_group(t):
                    def go():
                        ps = spsum.tile([128, 512], f32, tag="s",
                                        name=f"vps{t}")
                        for k in range(NK):
                            nc.tensor.matmul(
                                ps[:], xT_t[:, k, ts(t, 128)], wv_t[:, k],
                                start=(k == 0), stop=(k == NK - 1))
                        nc.vector.tensor_copy(
                            va4[:, t, :, 0:D],
                            ps.rearrange("p (h d) -> p h d", d=D))
                    return go

                def proj_group(t):
                    def go():
                        o_t = opool.tile([128, 1024], fp16, tag="o",
                                         name=f"o{t}")
                        ps = spsum.tile([128, 1024], f32, tag="s",
                                        name=f"pps{t}")
                        for n2 in range(2):
                            for kp in range(4):
                                nc.tensor.matmul(
                                    ps[:, ds(n2 * 512, 512)],
                                    yT[:, kp, ts(t, 128)],
                                    wp_t[:, kp, ds(n2 * 512, 512)],
                                    start=(kp == 0), stop=(kp == 3))
                        # halves pipeline: DMA of half 0 overlaps copy of half 1
                        for n2 in range(2):
                            sl = ds(n2 * 512, 512)
                            nc.vector.tensor_copy(o_t[:, sl], ps[:, sl])
                            nc.sync.dma_start(
                                out_d[ds(t * 128, 128), sl], o_t[:, sl])
                    return go

                deferred_norm = []

                def make_norm(avs, head, p, c, qsplit=False):
                    def go():
                        dn = npool.tile([1, 512], f32, tag="dn",
                                        name=f"dn{p}_{c}_{head}")
                        nc.vector.tensor_copy(dn[:], avs[D:D + 1, :])
                        rb = npool.tile([D, 512], f32, tag="rb",
                                        name=f"rb{p}_{c}_{head}")
                        nc.gpsimd.partition_broadcast(rb[:], dn[:])
                        rr = npool.tile([D, 512], f32, tag="rr",
                                        name=f"rr{p}_{c}_{head}")
                        nc.vector.reciprocal_approx_fast(out=rr[:], in_=rb[:])
                        if qsplit:
                            # emit per query-half so the tail projection can
                            # start on the first half sooner
                            for q2 in range(2):
                                sl = ds(q2 * 256, 256)
                                nc.vector.tensor_mul(
                                    yT[ds(D * head, D), p,
                                       ds(c * 512 + q2 * 256, 256)],
                                    avs[0:D, sl], rr[:, sl])
                        else:
                            nc.vector.tensor_mul(
                                yT[ds(D * head, D), p, ds(c * 512, 512)],
                                avs[0:D, :], rr[:])
                    return go

                # ---- attention: chunk-major with PE backlog interleave ----
                for c in range(NCH):
                    backlog = []
                    if c + 1 < NCH:
                        for t in range(4 * (c + 1), 4 * (c + 2)):
                            backlog.append(vproj_group(t))
                        for g in range(8):
                            backlog.append(qkproj_group(g // 2, g % 2, c + 1))
                    # output projection for chunk X interleaves two chunks
                    # later (X+2), where its yT norms are long complete; the
                    # ACT-saturated final chunk gets proj(c1) ungated plus
                    # proj(c2) in a second queue paced only after the
                    # (p=0, s==1) norm drain.
                    backlog2 = []
                    if c == 2:
                        for t in range(0, 4):
                            backlog.append(proj_group(t))
                    if c == 3:
                        for t in range(4, 8):
                            backlog.append(proj_group(t))
                        for t in range(8, 12):
                            backlog2.append(proj_group(t))
                    nblk = 4 * (c + 1)
                    nsuper = nblk // 2
                    total_slots = 4 * (nsuper + 2)
                    done_slots = 0
                    emitted = 0
                    emitted2 = 0
                    for p in range(NPAIR):
                        av_A = avpsum.tile([128, 512], f32, tag="av",
                                           name=f"avA{p}_{c}")
                        av_B = avpsum.tile([128, 512], f32, tag="av",
                                           name=f"avB{p}_{c}")
                        pend = {}
                        for s in range(nsuper + 2):
                            if s == 1 and deferred_norm:
                                for fn in deferred_norm:
                                    fn()
                                deferred_norm.clear()
                            if s < nsuper:
                                sA = spsum.tile([128, 1024], f32, tag="s",
                                                name=f"sA{p}_{c}_{s}")
                                sB = spsum.tile([128, 1024], f32, tag="s",
                                                name=f"sB{p}_{c}_{s}")
                                i = s - (nsuper - 2)
                                if i < 0:
                                    for half in (0, 1):
                                        tj = 2 * s + half
                                        nc.tensor.matmul(
                                            sA[:, ds(half * 512, 512)],
                                            qk[0:D, p, 1, ts(tj, 128)],
                                            qk[0:D, p, 0, ds(c * 512, 512)],
                                            start=True, stop=True)
                                        nc.tensor.matmul(
                                            sB[:, ds(half * 512, 512)],
                                            qk[D:128, p, 1, ts(tj, 128)],
                                            qk[D:128, p, 0, ds(c * 512, 512)],
                                            start=True, stop=True)
                                else:
                                    # band: columns [0, 128d) are fully masked
                                    # — compute only the live range
                                    for half in (0, 1):
                                        tj = 2 * s + half
                                        off = 128 * (2 * i + half)
                                        w = 512 - off
                                        nc.tensor.matmul(
                                            sA[:, ds(half * 512 + off, w)],
                                            qk[0:D, p, 1, ts(tj, 128)],
                                            qk[0:D, p, 0, ds(c * 512 + off, w)],
                                            start=True, stop=True)
                                        nc.tensor.matmul(
                                            sB[:, ds(half * 512 + off, w)],
                                            qk[D:128, p, 1, ts(tj, 128)],
                                            qk[D:128, p, 0, ds(c * 512 + off, w)],
                                            start=True, stop=True)
                                e_A = epool.tile([128, 1024], bf16, tag="e",
                                                 name=f"eA{p}_{c}_{s}")
                                e_B = epool.tile([128, 1024], bf16, tag="e",
                                                 name=f"eB{p}_{c}_{s}")
                                if i >= 0:
                                    for half in (0, 1):
                                        off = 128 * (2 * i + half)
                                        w = 512 - off
                                        sl = ds(half * 512 + off, w)
                                        nc.scalar.activation(e_A[:, sl], sA[:, sl],
                                                             Exp, scale=INV_SCALE)
                                        nc.scalar.activation(e_B[:, sl], sB[:, sl],
                                                             Exp, scale=INV_SCALE)
                                        # only the diagonal 128 columns of the
                                        # live range contain masked entries;
                                        # zero them with an in-place triangle
                                        # multiply.
                                        sd = ds(half * 512 + off, 128)
                                        nc.vector.tensor_mul(
                                            e_A[:, sd], e_A[:, sd], mask_t[:, 0:128])
                                        nc.vector.tensor_mul(
                                            e_B[:, sd], e_B[:, sd], mask_t[:, 0:128])
                                else:
                                    nc.scalar.activation(e_A[:], sA[:], Exp,
                                                         scale=INV_SCALE)
                                    nc.scalar.activation(e_B[:], sB[:], Exp,
                                                         scale=INV_SCALE)
                                pend[s] = (e_A, e_B)
                            if s >= 2:
                                e_A, e_B = pend.pop(s - 2)
                                s2 = s - 2
                                for half in (0, 1):
                                    tj = 2 * s2 + half
                                    dd = tj - (nblk - 4)
                                    # masked band columns [0,128d) of E are
                                    # zero after the mask multiply — skip them
                                    off = 128 * dd if dd > 0 else 0
                                    w = 512 - off
                                    nc.tensor.matmul(
                                        av_A[:, ds(off, w)],
                                        v_aug[:, tj, ds(2 * p * 128, 128)],
                                        e_A[:, ds(half * 512 + off, w)],
                                        start=(tj == 0), stop=(tj == nblk - 1))
                                    nc.tensor.matmul(
                                        av_B[:, ds(off, w)],
                                        v_aug[:, tj, ds((2 * p + 1) * 128, 128)],
                                        e_B[:, ds(half * 512 + off, w)],
                                        start=(tj == 0), stop=(tj == nblk - 1))
                            # pace the backlog across the chunk's superslots
                            done_slots += 1
                            want = -(-len(backlog) * done_slots // total_slots)
                            while emitted < want and emitted < len(backlog):
                                backlog[emitted]()
                                emitted += 1
                            if done_slots >= 2 and backlog2:
                                half = max(1, (total_slots - 1) // 2)
                                want2 = -(-len(backlog2) * (done_slots - 1)
                                          // half)
                                while emitted2 < want2 and emitted2 < len(backlog2):
                                    backlog2[emitted2]()
                                    emitted2 += 1
                        # stage av to SBUF with one copy so the PSUM banks free
                        # early; normalize from the staged copy.
                        for head, av in ((0, av_A), (1, av_B)):
                            avs = npool.tile([D + 1, 512], f32, tag="avs",
                                             bufs=4, name=f"avs{p}_{c}_{head}")
                            nc.vector.tensor_copy(avs[:], av[0:D + 1, :])
                            deferred_norm.append(make_norm(
                                avs, head, p, c,
                                qsplit=(p == NPAIR - 1 and c == NCH - 1)))

                for fn in deferred_norm:
                    fn()
                deferred_norm.clear()

                # ---- projection tail (last query chunk) ----
                for t in range(12, NT):
                    proj_group(t)()

    nc.compile()
    return nc


def _make_mask():
    # mask[p, j] = 1 iff j >= p: causal triangle in the first 128 cols of a
    # band live range, ones beyond.
    p = np.arange(128)[:, None]
    j = np.arange(512)[None, :]
    return (j >= p).astype(ml_dtypes.bfloat16)


def kernel(x: np.ndarray, W_attn: np.ndarray, W_proj: np.ndarray) -> np.ndarray:
    global LAST_RESULTS
    x = np.asarray(x, dtype=np.float32)
    W_attn = np.asarray(W_attn, dtype=np.float32)
    W_proj = np.asarray(W_proj, dtype=np.float32)

    nc = _cache.get("nc")
    if nc is None:
        nc = _build()
        _cache["nc"] = nc

    mask = _make_mask()
    xTs = [np.ascontiguousarray(x[b].T).astype(ml_dtypes.bfloat16) for b in range(B)]
    in_maps = []
    for cid in range(NCORES):
        b, hh = cid // 2, cid % 2
        qcols = W_attn[:, hh * 512:(hh + 1) * 512]
        kcols = W_attn[:, C + hh * 512:C + (hh + 1) * 512]
        wqk = np.concatenate([qcols, kcols], axis=1)                  # [1024, 1024]
        # pack to [p, ko, pair, m2, mm] -> [128, NK, 1024]
        wqk_pack = np.ascontiguousarray(
            wqk.reshape(NK, 128, 2, NPAIR, 128).transpose(1, 0, 3, 2, 4)
            .reshape(128, NK, NPAIR * 256)
        ).astype(ml_dtypes.bfloat16)
        wv = np.ascontiguousarray(
            W_attn[:, 2 * C + hh * 512:2 * C + (hh + 1) * 512]
        ).astype(ml_dtypes.bfloat16)
        wp = np.ascontiguousarray(W_proj[hh * 512:(hh + 1) * 512, :]).astype(np.float16)
        in_maps.append({
            "xT": xTs[b], "wqk": wqk_pack, "wv": wv, "wp": wp, "mask": mask,
        })

    res = run_bass_kernel_spmd(nc, in_maps, core_ids=list(range(NCORES)))
    LAST_RESULTS = res
    parts = [res.results[cid]["out"].astype(np.float32) for cid in range(NCORES)]
    out = np.stack([parts[2 * b] + parts[2 * b + 1] for b in range(B)], axis=0)
    return np.ascontiguousarray(out, dtype=np.float32)


# revision 18
# speedup vs baseline: 2.1461x; 1.0044x over previous
"""Batched causal self-attention (B=4, T=2048, C=1024, H=16) on 8 trn2 NeuronCores.

Sharding: data-parallel over B (4) x tensor-parallel over head-halves (2).
Core c handles batch b=c//2, heads [hh*8, hh*8+8) with hh=c%2. Each core
computes its qkv projection slice, causal attention for its 8 heads, and a
partial output projection (512 rows of W_proj); the host sums the two
partials per batch (the TP all-reduce).

v2: chunk-major schedule. The kernel is PE-streaming-bound (~225us of
matmul columns at 2.4GHz); everything else is arranged to keep the PE
continuously fed:
  - startup: k-outer qkv projection waves paced by the xT tile DMAs, so the
    PE starts ~4us in instead of waiting for the full 4MB activation load.
  - chunk-major attention (for c: for pair:) so the output projection for
    chunk c-1 is ready to interleave during chunk c instead of piling up
    behind the last pair (pair-major left an 8us serialized tail).
  - superslots of two key blocks: S^T via two K=64 matmuls per head pair
    (PE row groups), exp on ACT (scale=1/8 folded, no max-subtraction:
    scores ~N(0,0.4^2)), causal mask as an in-place [128,128] multiply on
    only the diagonal block (the rest of the band needs no mask), AV psum
    accumulation with v_aug (64 v-dims + ones column for the denominator +
    zero pad to 128 for FWL) running 2 superslots behind S.
  - independent PE work (next chunk's qkv projection slices, v tiles, and
    chunk c-1's output projection) is emitted into the superslot stream via
    a paced backlog to cover the exp shadow.
  - DMA triggers cost ~0.7us each on the sync queue, so inputs are
    consolidated (wqk packed host-side into one tensor, 2 triggers) and
    outputs merged to one [128,1024] DMA per row tile.
"""

import numpy as np
import ml_dtypes

import concourse.bass as bass
import concourse.mybir as mybir
import concourse.tile as tile
from concourse import bacc
from concourse.bass import ds, ts
from concourse.bass_utils import run_bass_kernel_spmd

B, T, C, H = 4, 2048, 1024, 16
D = 64
NCORES = 8
NPAIR = 4              # head pairs per core (8 heads)
NK = C // 128          # 8 contraction tiles over C
NT = T // 128          # 16 tiles over T
NCH = T // 512         # 4 query chunks
INV_SCALE = 0.125      # 1 / sqrt(C // H)

f32 = mybir.dt.float32
bf16 = mybir.dt.bfloat16
fp16 = mybir.dt.float16

_cache = {}
LAST_RESULTS = None    # test harness reads exec_time_ns from here

# If the caller sets BASS_TRACE=1, run_bass_kernel_spmd imports
# antenv.axon_hooks, which some container images don't ship. Provide a stub
# so tracing degrades gracefully instead of raising ImportError.
try:
    import antenv.axon_hooks  # noqa: F401
except ImportError:
    import sys as _sys
    import types as _types

    _m = _types.ModuleType("antenv.axon_hooks")
    _m._hook = None
    _m.set_axon_ntff_profile_hook = lambda h: setattr(_m, "_hook", h)
    _m.get_axon_ntff_profile_hook = lambda: _m._hook
    _sys.modules["antenv.axon_hooks"] = _m


def _build():
    nc = bacc.Bacc("TRN2", target_bir_lowering=False, debug=False)
    xT_d = nc.dram_tensor("xT", [C, T], bf16, kind="ExternalInput").ap()
    # host-packed: [p, ko, pair*2+m2 flattened to 1024 cols]
    wqk_d = nc.dram_tensor("wqk", [128, NK, NPAIR * 256], bf16,
                           kind="ExternalInput").ap()
    wv_d = nc.dram_tensor("wv", [C, 512], bf16, kind="ExternalInput").ap()
    wp_d = nc.dram_tensor("wp", [512, C], fp16, kind="ExternalInput").ap()
    mask_d = nc.dram_tensor("mask", [128, 512], bf16, kind="ExternalInput").ap()
    out_d = nc.dram_tensor("out", [T, C], fp16, kind="ExternalOutput").ap()

    Exp = mybir.ActivationFunctionType.Exp

    with tile.TileContext(nc) as tc:
        with tc.tile_pool(name="persist", bufs=1) as persist:
            xT_t = persist.tile([128, NK, T], bf16, tag="xT")
            wqk_t = persist.tile([128, NK, NPAIR * 256], bf16, tag="wqk")
            wv_t = persist.tile([128, NK, 512], bf16, tag="wv")
            wp_t = persist.tile([128, 4, C], fp16, tag="wp")
            mask_t = persist.tile([128, 512], bf16, tag="mask")
            # per-head blocks padded to 128 cols (v[0:64] | ones at 64 | zeros)
            # so the AV matmul's weight load is exactly 128 columns -> FWL.
            v_aug = persist.tile([128, NT, 8 * 128], bf16, tag="vaug")
            qk = persist.tile([128, NPAIR, 2, T], bf16, tag="qk")
            yT = persist.tile([128, NPAIR, T], fp16, tag="yT")

            # ---- input DMAs, ordered by first use. The startup waves only
            # ---- stream query-chunk-0 columns of xT, so xT is split: cols
            # ---- [0,512) per k-tile early (paces wave A), the rest after wv.
            nc.sync.dma_start(wqk_t[:, 0:1], wqk_d[:, 0:1])
            nc.sync.dma_start(xT_t[:, 0, 0:512], xT_d[ds(0, 128), 0:512])
            nc.sync.dma_start(wqk_t[:, 1:2], wqk_d[:, 1:2])
            nc.sync.dma_start(xT_t[:, 1, 0:512], xT_d[ds(128, 128), 0:512])
            for i in range(1, 4):
                nc.sync.dma_start(wqk_t[:, 2 * i:2 * i + 2],
                                  wqk_d[:, 2 * i:2 * i + 2])
                for k in (2 * i, 2 * i + 1):
                    nc.sync.dma_start(xT_t[:, k, 0:512],
                                      xT_d[ds(k * 128, 128), 0:512])
            nc.sync.dma_start(
                wv_t[:], wv_d.rearrange("(k p) m -> p k m", p=128))
            for k in range(NK):
                nc.sync.dma_start(xT_t[:, k, 512:T],
                                  xT_d[ds(k * 128, 128), 512:T])
            nc.sync.dma_start(mask_t[:], mask_d)
            nc.sync.dma_start(
                wp_t[:], wp_d.rearrange("(kp p) m -> p kp m", p=128))

            # v_aug pad init on DVE (idle during the DMA ramp): zeros in
            # cols [D,128) of each head block, ones column at D.
            va4 = v_aug.rearrange("p n (h e) -> p n h e", e=128)
            nc.vector.memset(va4[:, :, :, D:128].bitcast(mybir.dt.uint16), 0)
            nc.vector.memset(
                va4[:, :, :, D:D + 1].bitcast(mybir.dt.uint16), 0x3F80)
            ones_row = persist.tile([1, D], bf16, tag="ones")
            nc.vector.memset(ones_row.bitcast(mybir.dt.uint16), 0x3F80)

            # ---- startup waves: qk projection for chunk 0 (k-outer, DMA
            # ---- paced), then v tiles 0..3.
            with tc.tile_pool(name="spool", bufs=8, space="PSUM") as spool:
                wave = [
                    spool.tile([128, 512], f32, tag="w", name=f"waveA{g}")
                    for g in range(8)
                ]
                for k in range(NK):
                    for g in range(8):
                        p, m2 = g // 2, g % 2
                        nc.tensor.matmul(
                            wave[g][:],
                            wqk_t[:, k, ds(p * 256 + m2 * 128, 128)],
                            xT_t[:, k, 0:512],
                            start=(k == 0), stop=(k == NK - 1))
                for g in range(8):
                    p, m2 = g // 2, g % 2
                    nc.scalar.copy(qk[:, p, m2, 0:512], wave[g][:])
                for t in range(4):
                    vps = spool.tile([128, 512], f32, tag="w", name=f"waveV{t}")
                    for k in range(NK):
                        nc.tensor.matmul(
                            vps[:], xT_t[:, k, ts(t, 128)], wv_t[:, k],
                            start=(k == 0), stop=(k == NK - 1))
                    nc.vector.tensor_copy(
                        va4[:, t, :, 0:D],
                        vps.rearrange("p (h d) -> p h d", d=D))

            with (
                tc.tile_pool(name="epool", bufs=8) as epool,
                tc.tile_pool(name="npool", bufs=2) as npool,
                tc.tile_pool(name="opool", bufs=2) as opool,
                tc.tile_pool(name="spsum", bufs=3, space="PSUM") as spsum,
                tc.tile_pool(name="avpsum", bufs=2, space="PSUM") as avpsum,
            ):
                # ---- interleavable PE work units ----
                def qkproj_group(p, m2, n):
                    def go():
                        ps = spsum.tile([128, 512], f32, tag="s",
                                        name=f"qkps{p}_{m2}_{n}")
                        for k in range(NK):
                            nc.tensor.matmul(
                                ps[:], wqk_t[:, k, ds(p * 256 + m2 * 128, 128)],
                                xT_t[:, k, ds(n * 512, 512)],
                                start=(k == 0), stop=(k == NK - 1))
                        nc.vector.tensor_copy(
                            qk[:, p, m2, ds(n * 512, 512)], ps[:])
                    return go

                def vproj_group(t):
                    def go():
                        ps = spsum.tile([128, 512], f32, tag="s",
                                        name=f"vps{t}")
                        for k in range(NK):
                            nc.tensor.matmul(
                                ps[:], xT_t[:, k, ts(t, 128)], wv_t[:, k],
                                start=(k == 0), stop=(k == NK - 1))
                        nc.vector.tensor_copy(
                            va4[:, t, :, 0:D],
                            ps.rearrange("p (h d) -> p h d", d=D))
                    return go

                def proj_group(t):
                    def go():
                        o_t = opool.tile([128, 1024], fp16, tag="o",
                                         name=f"o{t}")
                        ps = spsum.tile([128, 1024], f32, tag="s",
                                        name=f"pps{t}")
                        for n2 in range(2):
                            for kp in range(4):
                                nc.tensor.matmul(
                                    ps[:, ds(n2 * 512, 512)],
                                    yT[:, kp, ts(t, 128)],
                                    wp_t[:, kp, ds(n2 * 512, 512)],
                                    start=(kp == 0), stop=(kp == 3))
                        # halves pipeline: DMA of half 0 overlaps copy of half 1
                        for n2 in range(2):
                            sl = ds(n2 * 512, 512)
                            nc.vector.tensor_copy(o_t[:, sl], ps[:, sl])
                            nc.sync.dma_start(
                                out_d[ds(t * 128, 128), sl], o_t[:, sl])
                    return go

                deferred_norm = []

                def make_norm(avs, head, p, c, qsplit=False):
                    def go():
                        dn = npool.tile([1, 512], f32, tag="dn",
                                        name=f"dn{p}_{c}_{head}")
                        nc.vector.tensor_copy(dn[:], avs[D:D + 1, :])
                        rb = npool.tile([D, 512], f32, tag="rb",
                                        name=f"rb{p}_{c}_{head}")
                        nc.gpsimd.partition_broadcast(rb[:], dn[:])
                        rr = npool.tile([D, 512], f32, tag="rr",
                                        name=f"rr{p}_{c}_{head}")
                        nc.vector.reciprocal_approx_fast(out=rr[:], in_=rb[:])
                        if qsplit:
                            # emit per query-half so the tail projection can
                            # start on the first half sooner
                            for q2 in range(2):
                                sl = ds(q2 * 256, 256)
                                nc.vector.tensor_mul(
                                    yT[ds(D * head, D), p,
                                       ds(c * 512 + q2 * 256, 256)],
                                    avs[0:D, sl], rr[:, sl])
                        else:
                            nc.vector.tensor_mul(
                                yT[ds(D * head, D), p, ds(c * 512, 512)],
                                avs[0:D, :], rr[:])
                    return go

                # ---- attention: chunk-major with PE backlog interleave ----
                for c in range(NCH):
                    backlog = []
                    if c + 1 < NCH:
                        for t in range(4 * (c + 1), 4 * (c + 2)):
                            backlog.append(vproj_group(t))
                        for g in range(8):
                            backlog.append(qkproj_group(g // 2, g % 2, c + 1))
                    # output projection for chunk X interleaves two chunks
                    # later (X+2), where its yT norms are long complete; the
                    # ACT-saturated final chunk gets proj(c1) ungated plus
                    # proj(c2) in a second queue paced only after the
                    # (p=0, s==1) norm drain.
                    backlog2 = []
                    if c == 2:
                        for t in range(0, 4):
                            backlog.append(proj_group(t))
                    if c == 3:
                        for t in range(4, 8):
                            backlog.append(proj_group(t))
                        for t in range(8, 12):
                            backlog2.append(proj_group(t))
                    nblk = 4 * (c + 1)
                    nsuper = nblk // 2
                    total_slots = 4 * (nsuper + 2)
                    done_slots = 0
                    emitted = 0
                    emitted2 = 0
                    leftover = (backlog, backlog2) if c == NCH - 1 else None
                    for p in range(NPAIR):
                        av_A = avpsum.tile([128, 512], f32, tag="av",
                                           name=f"avA{p}_{c}")
                        av_B = avpsum.tile([128, 512], f32, tag="av",
                                           name=f"avB{p}_{c}")
                        pend = {}
                        for s in range(nsuper + 2):
                            if s == 1 and deferred_norm:
                                for fn in deferred_norm:
                                    fn()
                                deferred_norm.clear()
                            if s < nsuper:
                                sA = spsum.tile([128, 1024], f32, tag="s",
                                                name=f"sA{p}_{c}_{s}")
                                sB = spsum.tile([128, 1024], f32, tag="s",
                                                name=f"sB{p}_{c}_{s}")
                                i = s - (nsuper - 2)
                                if i < 0:
                                    for half in (0, 1):
                                        tj = 2 * s + half
                                        nc.tensor.matmul(
                                            sA[:, ds(half * 512, 512)],
                                            qk[0:D, p, 1, ts(tj, 128)],
                                            qk[0:D, p, 0, ds(c * 512, 512)],
                                            start=True, stop=True)
                                        nc.tensor.matmul(
                                            sB[:, ds(half * 512, 512)],
                                            qk[D:128, p, 1, ts(tj, 128)],
                                            qk[D:128, p, 0, ds(c * 512, 512)],
                                            start=True, stop=True)
                                else:
                                    # band: columns [0, 128d) are fully masked
                                    # — compute only the live range
                                    for half in (0, 1):
                                        tj = 2 * s + half
                                        off = 128 * (2 * i + half)
                                        w = 512 - off
                                        nc.tensor.matmul(
                                            sA[:, ds(half * 512 + off, w)],
                                            qk[0:D, p, 1, ts(tj, 128)],
                                            qk[0:D, p, 0, ds(c * 512 + off, w)],
                                            start=True, stop=True)
                                        nc.tensor.matmul(
                                            sB[:, ds(half * 512 + off, w)],
                                            qk[D:128, p, 1, ts(tj, 128)],
                                            qk[D:128, p, 0, ds(c * 512 + off, w)],
                                            start=True, stop=True)
                                e_A = epool.tile([128, 1024], bf16, tag="e",
                                                 name=f"eA{p}_{c}_{s}")
                                e_B = epool.tile([128, 1024], bf16, tag="e",
                                                 name=f"eB{p}_{c}_{s}")
                                if i >= 0:
                                    for half in (0, 1):
                                        off = 128 * (2 * i + half)
                                        w = 512 - off
                                        sl = ds(half * 512 + off, w)
                                        nc.scalar.activation(e_A[:, sl], sA[:, sl],
                                                             Exp, scale=INV_SCALE)
                                        nc.scalar.activation(e_B[:, sl], sB[:, sl],
                                                             Exp, scale=INV_SCALE)
                                        # only the diagonal 128 columns of the
                                        # live range contain masked entries;
                                        # zero them with an in-place triangle
                                        # multiply.
                                        sd = ds(half * 512 + off, 128)
                                        nc.vector.tensor_mul(
                                            e_A[:, sd], e_A[:, sd], mask_t[:, 0:128])
                                        nc.vector.tensor_mul(
                                            e_B[:, sd], e_B[:, sd], mask_t[:, 0:128])
                                else:
                                    nc.scalar.activation(e_A[:], sA[:], Exp,
                                                         scale=INV_SCALE)
                                    nc.scalar.activation(e_B[:], sB[:], Exp,
                                                         scale=INV_SCALE)
                                pend[s] = (e_A, e_B)
                            if s >= 2:
                                e_A, e_B = pend.pop(s - 2)
                                s2 = s - 2
                                for half in (0, 1):
                                    tj = 2 * s2 + half
                                    dd = tj - (nblk - 4)
                                    # masked band columns [0,128d) of E are
                                    # zero after the mask multiply — skip them
                                    off = 128 * dd if dd > 0 else 0
                                    w = 512 - off
                                    nc.tensor.matmul(
                                        av_A[:, ds(off, w)],
                                        v_aug[:, tj, ds(2 * p * 128, 128)],
                                        e_A[:, ds(half * 512 + off, w)],
                                        start=(tj == 0), stop=(tj == nblk - 1))
                                    nc.tensor.matmul(
                                        av_B[:, ds(off, w)],
                                        v_aug[:, tj, ds((2 * p + 1) * 128, 128)],
                                        e_B[:, ds(half * 512 + off, w)],
                                        start=(tj == 0), stop=(tj == nblk - 1))
                            # pace the backlog across the chunk's superslots;
                            # in the final chunk, hold the last items back so the
                            # closing norm chain isn't queued behind their DVE
                            # copies (emitted after the drain below instead).
                            done_slots += 1
                            cap = len(backlog) - 2 if c == NCH - 1 else len(backlog)
                            want = min(cap, -(-len(backlog) * done_slots
                                              // total_slots))
                            while emitted < want and emitted < len(backlog):
                                backlog[emitted]()
                                emitted += 1
                            if done_slots >= 2 and backlog2:
                                half = max(1, (total_slots - 1) // 2)
                                cap2 = (len(backlog2) - 2 if c == NCH - 1
                                        else len(backlog2))
                                want2 = min(cap2, -(-len(backlog2) * (done_slots - 1)
                                                    // half))
                                while emitted2 < want2 and emitted2 < len(backlog2):
                                    backlog2[emitted2]()
                                    emitted2 += 1
                        # stage av to SBUF with one copy so the PSUM banks free
                        # early; normalize from the staged copy.
                        for head, av in ((0, av_A), (1, av_B)):
                            avs = npool.tile([D + 1, 512], f32, tag="avs",
                                             bufs=4, name=f"avs{p}_{c}_{head}")
                            nc.vector.tensor_copy(avs[:], av[0:D + 1, :])
                            deferred_norm.append(make_norm(
                                avs, head, p, c,
                                qsplit=(p == NPAIR - 1 and c == NCH - 1)))

                for fn in deferred_norm:
                    fn()
                deferred_norm.clear()

                # held-back final-chunk backlog items run after the norm chain
                if leftover is not None:
                    bl, bl2 = leftover
                    for item in bl[emitted:]:
                        item()
                    for item in bl2[emitted2:]:
                        item()

                # ---- projection tail (last query chunk) ----
                for t in range(12, NT):
                    proj_group(t)()

    nc.compile()
    return nc


def _make_mask():
    # mask[p, j] = 1 iff j >= p: causal triangle in the first 128 cols of a
    # band live range, ones beyond.
    p = np.arange(128)[:, None]
    j = np.arange(512)[None, :]
    return (j >= p).astype(ml_dtypes.bfloat16)


def kernel(x: np.ndarray, W_attn: np.ndarray, W_proj: np.ndarray) -> np.ndarray:
    global LAST_RESULTS
    x = np.asarray(x, dtype=np.float32)
    W_attn = np.asarray(W_attn, dtype=np.float32)
    W_proj = np.asarray(W_proj, dtype=np.float32)

    nc = _cache.get("nc")
    if nc is None:
        nc = _build()
        _cache["nc"] = nc

    mask = _make_mask()
    xTs = [np.ascontiguousarray(x[b].T).astype(ml_dtypes.bfloat16) for b in range(B)]
    in_maps = []
    for cid in range(NCORES):
        b, hh = cid // 2, cid % 2
        qcols = W_attn[:, hh * 512:(hh + 1) * 512]
        kcols = W_attn[:, C + hh * 512:C + (hh + 1) * 512]
        wqk = np.concatenate([qcols, kcols], axis=1)                  # [1024, 1024]
        # pack to [p, ko, pair, m2, mm] -> [128, NK, 1024]
        wqk_pack = np.ascontiguousarray(
            wqk.reshape(NK, 128, 2, NPAIR, 128).transpose(1, 0, 3, 2, 4)
            .reshape(128, NK, NPAIR * 256)
        ).astype(ml_dtypes.bfloat16)
        wv = np.ascontiguousarray(
            W_attn[:, 2 * C + hh * 512:2 * C + (hh + 1) * 512]
        ).astype(ml_dtypes.bfloat16)
        wp = np.ascontiguousarray(W_proj[hh * 512:(hh + 1) * 512, :]).astype(np.float16)
        in_maps.append({
            "xT": xTs[b], "wqk": wqk_pack, "wv": wv, "wp": wp, "mask": mask,
        })

    res = run_bass_kernel_spmd(nc, in_maps, core_ids=list(range(NCORES)))
    LAST_RESULTS = res
    parts = [res.results[cid]["out"].astype(np.float32) for cid in range(NCORES)]
    out = np.stack([parts[2 * b] + parts[2 * b + 1] for b in range(B)], axis=0)
    return np.ascontiguousarray(out, dtype=np.float32)


# revision 19
# speedup vs baseline: 2.2079x; 1.0288x over previous
"""Batched causal self-attention (B=4, T=2048, C=1024, H=16) on 8 trn2 NeuronCores.

Sharding: data-parallel over B (4) x tensor-parallel over head-halves (2).
Core c handles batch b=c//2, heads [hh*8, hh*8+8) with hh=c%2. Each core
computes its qkv projection slice, causal attention for its 8 heads, and a
partial output projection (512 rows of W_proj); the host sums the two
partials per batch (the TP all-reduce).

v2: chunk-major schedule. The kernel is PE-streaming-bound (~225us of
matmul columns at 2.4GHz); everything else is arranged to keep the PE
continuously fed:
  - startup: k-outer qkv projection waves paced by the xT tile DMAs, so the
    PE starts ~4us in instead of waiting for the full 4MB activation load.
  - chunk-major attention (for c: for pair:) so the output projection for
    chunk c-1 is ready to interleave during chunk c instead of piling up
    behind the last pair (pair-major left an 8us serialized tail).
  - superslots of two key blocks: S^T via two K=64 matmuls per head pair
    (PE row groups), exp on ACT (scale=1/8 folded, no max-subtraction:
    scores ~N(0,0.4^2)), causal mask as an in-place [128,128] multiply on
    only the diagonal block (the rest of the band needs no mask), AV psum
    accumulation with v_aug (64 v-dims + ones column for the denominator +
    zero pad to 128 for FWL) running 2 superslots behind S.
  - independent PE work (next chunk's qkv projection slices, v tiles, and
    chunk c-1's output projection) is emitted into the superslot stream via
    a paced backlog to cover the exp shadow.
  - DMA triggers cost ~0.7us each on the sync queue, so inputs are
    consolidated (wqk packed host-side into one tensor, 2 triggers) and
    outputs merged to one [128,1024] DMA per row tile.
"""

import numpy as np
import ml_dtypes

import concourse.bass as bass
import concourse.mybir as mybir
import concourse.tile as tile
from concourse import bacc
from concourse.bass import ds, ts
from concourse.bass_utils import run_bass_kernel_spmd

B, T, C, H = 4, 2048, 1024, 16
D = 64
NCORES = 8
NPAIR = 4              # head pairs per core (8 heads)
NK = C // 128          # 8 contraction tiles over C
NT = T // 128          # 16 tiles over T
NCH = T // 512         # 4 query chunks
INV_SCALE = 0.125      # 1 / sqrt(C // H)

f32 = mybir.dt.float32
bf16 = mybir.dt.bfloat16
fp16 = mybir.dt.float16

_cache = {}
LAST_RESULTS = None    # test harness reads exec_time_ns from here

# If the caller sets BASS_TRACE=1, run_bass_kernel_spmd imports
# antenv.axon_hooks, which some container images don't ship. Provide a stub
# so tracing degrades gracefully instead of raising ImportError.
try:
    import antenv.axon_hooks  # noqa: F401
except ImportError:
    import sys as _sys
    import types as _types

    _m = _types.ModuleType("antenv.axon_hooks")
    _m._hook = None
    _m.set_axon_ntff_profile_hook = lambda h: setattr(_m, "_hook", h)
    _m.get_axon_ntff_profile_hook = lambda: _m._hook
    _sys.modules["antenv.axon_hooks"] = _m


def _build():
    nc = bacc.Bacc("TRN2", target_bir_lowering=False, debug=False)
    xT_d = nc.dram_tensor("xT", [C, T], bf16, kind="ExternalInput").ap()
    # host-packed: [p, ko, pair*2+m2 flattened to 1024 cols]
    wqk_d = nc.dram_tensor("wqk", [128, NK, NPAIR * 256], bf16,
                           kind="ExternalInput").ap()
    wv_d = nc.dram_tensor("wv", [C, 512], bf16, kind="ExternalInput").ap()
    wp_d = nc.dram_tensor("wp", [512, C], fp16, kind="ExternalInput").ap()
    mask_d = nc.dram_tensor("mask", [128, 512], bf16, kind="ExternalInput").ap()
    out_d = nc.dram_tensor("out", [T, C], fp16, kind="ExternalOutput").ap()

    Exp = mybir.ActivationFunctionType.Exp

    with tile.TileContext(nc) as tc:
        with tc.tile_pool(name="persist", bufs=1) as persist:
            xT_t = persist.tile([128, NK, T], bf16, tag="xT")
            wqk_t = persist.tile([128, NK, NPAIR * 256], bf16, tag="wqk")
            wv_t = persist.tile([128, NK, 512], bf16, tag="wv")
            wp_t = persist.tile([128, 4, C], fp16, tag="wp")
            mask_t = persist.tile([128, 512], bf16, tag="mask")
            # per-head blocks padded to 128 cols (v[0:64] | ones at 64 | zeros)
            # so the AV matmul's weight load is exactly 128 columns -> FWL.
            v_aug = persist.tile([128, NT, 8 * 128], bf16, tag="vaug")
            qk = persist.tile([128, NPAIR, 2, T], bf16, tag="qk")
            yT = persist.tile([128, NPAIR, T], fp16, tag="yT")

            # ---- input DMAs, ordered by first use. The startup waves only
            # ---- stream query-chunk-0 columns of xT, so xT is split: cols
            # ---- [0,512) per k-tile early (paces wave A), the rest after wv.
            nc.sync.dma_start(wqk_t[:, 0:1], wqk_d[:, 0:1])
            nc.sync.dma_start(xT_t[:, 0, 0:512], xT_d[ds(0, 128), 0:512])
            nc.sync.dma_start(wqk_t[:, 1:2], wqk_d[:, 1:2])
            nc.sync.dma_start(xT_t[:, 1, 0:512], xT_d[ds(128, 128), 0:512])
            for i in range(1, 4):
                nc.sync.dma_start(wqk_t[:, 2 * i:2 * i + 2],
                                  wqk_d[:, 2 * i:2 * i + 2])
                for k in (2 * i, 2 * i + 1):
                    nc.sync.dma_start(xT_t[:, k, 0:512],
                                      xT_d[ds(k * 128, 128), 0:512])
            nc.sync.dma_start(
                wv_t[:], wv_d.rearrange("(k p) m -> p k m", p=128))
            for k in range(NK):
                nc.sync.dma_start(xT_t[:, k, 512:T],
                                  xT_d[ds(k * 128, 128), 512:T])
            nc.sync.dma_start(mask_t[:], mask_d)
            nc.sync.dma_start(
                wp_t[:], wp_d.rearrange("(kp p) m -> p kp m", p=128))

            # v_aug pad init on DVE (idle during the DMA ramp): zeros in
            # cols [D,128) of each head block, ones column at D.
            va4 = v_aug.rearrange("p n (h e) -> p n h e", e=128)
            nc.vector.memset(va4[:, :, :, D:128].bitcast(mybir.dt.uint16), 0)
            nc.vector.memset(
                va4[:, :, :, D:D + 1].bitcast(mybir.dt.uint16), 0x3F80)
            ones_row = persist.tile([1, D], bf16, tag="ones")
            nc.vector.memset(ones_row.bitcast(mybir.dt.uint16), 0x3F80)

            # ---- startup waves: qk projection for chunk 0 (k-outer, DMA
            # ---- paced), then v tiles 0..3.
            with tc.tile_pool(name="spool", bufs=8, space="PSUM") as spool:
                wave = [
                    spool.tile([128, 512], f32, tag="w", name=f"waveA{g}")
                    for g in range(8)
                ]
                for k in range(NK):
                    for g in range(8):
                        p, m2 = g // 2, g % 2
                        nc.tensor.matmul(
                            wave[g][:],
                            wqk_t[:, k, ds(p * 256 + m2 * 128, 128)],
                            xT_t[:, k, 0:512],
                            start=(k == 0), stop=(k == NK - 1))
                for g in range(8):
                    p, m2 = g // 2, g % 2
                    nc.scalar.copy(qk[:, p, m2, 0:512], wave[g][:])
                for t in range(4):
                    vps = spool.tile([128, 512], f32, tag="w", name=f"waveV{t}")
                    for k in range(NK):
                        nc.tensor.matmul(
                            vps[:], xT_t[:, k, ts(t, 128)], wv_t[:, k],
                            start=(k == 0), stop=(k == NK - 1))
                    nc.vector.tensor_copy(
                        va4[:, t, :, 0:D],
                        vps.rearrange("p (h d) -> p h d", d=D))

            with (
                tc.tile_pool(name="epool", bufs=8) as epool,
                tc.tile_pool(name="npool", bufs=2) as npool,
                tc.tile_pool(name="opool", bufs=2) as opool,
                tc.tile_pool(name="spsum", bufs=3, space="PSUM") as spsum,
                tc.tile_pool(name="avpsum", bufs=2, space="PSUM") as avpsum,
            ):
                # ---- interleavable PE work units ----
                def qkproj_group(p, m2, n):
                    def go():
                        ps = spsum.tile([128, 512], f32, tag="s",
                                        name=f"qkps{p}_{m2}_{n}")
                        for k in range(NK):
                            nc.tensor.matmul(
                                ps[:], wqk_t[:, k, ds(p * 256 + m2 * 128, 128)],
                                xT_t[:, k, ds(n * 512, 512)],
                                start=(k == 0), stop=(k == NK - 1))
                        nc.vector.tensor_copy(
                            qk[:, p, m2, ds(n * 512, 512)], ps[:])
                    return go

                def vproj_group(t):
                    def go():
                        ps = spsum.tile([128, 512], f32, tag="s",
                                        name=f"vps{t}")
                        for k in range(NK):
                            nc.tensor.matmul(
                                ps[:], xT_t[:, k, ts(t, 128)], wv_t[:, k],
                                start=(k == 0), stop=(k == NK - 1))
                        nc.vector.tensor_copy(
                            va4[:, t, :, 0:D],
                            ps.rearrange("p (h d) -> p h d", d=D))
                    return go

                def proj_group(t):
                    def go():
                        o_t = opool.tile([128, 1024], fp16, tag="o",
                                         name=f"o{t}")
                        ps = spsum.tile([128, 1024], f32, tag="s",
                                        name=f"pps{t}")
                        for n2 in range(2):
                            for kp in range(4):
                                nc.tensor.matmul(
                                    ps[:, ds(n2 * 512, 512)],
                                    yT[:, kp, ts(t, 128)],
                                    wp_t[:, kp, ds(n2 * 512, 512)],
                                    start=(kp == 0), stop=(kp == 3))
                        # halves pipeline: DMA of half 0 overlaps copy of half 1
                        for n2 in range(2):
                            sl = ds(n2 * 512, 512)
                            nc.vector.tensor_copy(o_t[:, sl], ps[:, sl])
                            nc.sync.dma_start(
                                out_d[ds(t * 128, 128), sl], o_t[:, sl])
                    return go

                deferred_norm = []

                def make_norm(avs, head, p, c, qsplit=False):
                    def go():
                        dn = npool.tile([1, 512], f32, tag="dn",
                                        name=f"dn{p}_{c}_{head}")
                        nc.vector.tensor_copy(dn[:], avs[D:D + 1, :])
                        rb = npool.tile([D, 512], f32, tag="rb",
                                        name=f"rb{p}_{c}_{head}")
                        nc.gpsimd.partition_broadcast(rb[:], dn[:])
                        rr = npool.tile([D, 512], f32, tag="rr",
                                        name=f"rr{p}_{c}_{head}")
                        nc.vector.reciprocal_approx_fast(out=rr[:], in_=rb[:])
                        if qsplit:
                            # emit per query-half so the tail projection can
                            # start on the first half sooner
                            for q2 in range(2):
                                sl = ds(q2 * 256, 256)
                                nc.vector.tensor_mul(
                                    yT[ds(D * head, D), p,
                                       ds(c * 512 + q2 * 256, 256)],
                                    avs[0:D, sl], rr[:, sl])
                        else:
                            nc.vector.tensor_mul(
                                yT[ds(D * head, D), p, ds(c * 512, 512)],
                                avs[0:D, :], rr[:])
                    return go

                # ---- attention: chunk-major with PE backlog interleave ----
                for c in range(NCH):
                    backlog = []
                    if c + 1 < NCH:
                        for t in range(4 * (c + 1), 4 * (c + 2)):
                            backlog.append(vproj_group(t))
                        for g in range(8):
                            backlog.append(qkproj_group(g // 2, g % 2, c + 1))
                    # output projection for chunk X interleaves two chunks
                    # later (X+2), where its yT norms are long complete; the
                    # ACT-saturated final chunk gets proj(c1) ungated plus
                    # proj(c2) in a second queue paced only after the
                    # (p=0, s==1) norm drain.
                    backlog2 = []
                    if c == 2:
                        for t in range(0, 4):
                            backlog.append(proj_group(t))
                    if c == 3:
                        for t in range(4, 8):
                            backlog.append(proj_group(t))
                        for t in range(8, 12):
                            backlog2.append(proj_group(t))
                    nblk = 4 * (c + 1)
                    nsuper = nblk // 2
                    total_slots = 4 * (nsuper + 2)
                    done_slots = 0
                    emitted = 0
                    emitted2 = 0
                    leftover = (backlog, backlog2) if c == NCH - 1 else None
                    for p in range(NPAIR):
                        av_A = avpsum.tile([128, 512], f32, tag="av",
                                           name=f"avA{p}_{c}")
                        av_B = avpsum.tile([128, 512], f32, tag="av",
                                           name=f"avB{p}_{c}")
                        pend = {}
                        for s in range(nsuper + 2):
                            if s == 1 and deferred_norm:
                                for fn in deferred_norm:
                                    fn()
                                deferred_norm.clear()
                            if s >= 2:
                                e_A, e_B = pend.pop(s - 2)
                                s2 = s - 2
                                for half in (0, 1):
                                    tj = 2 * s2 + half
                                    dd = tj - (nblk - 4)
                                    # masked band columns [0,128d) of E are
                                    # zero after the mask multiply — skip them
                                    off = 128 * dd if dd > 0 else 0
                                    w = 512 - off
                                    nc.tensor.matmul(
                                        av_A[:, ds(off, w)],
                                        v_aug[:, tj, ds(2 * p * 128, 128)],
                                        e_A[:, ds(half * 512 + off, w)],
                                        start=(tj == 0), stop=(tj == nblk - 1))
                                    nc.tensor.matmul(
                                        av_B[:, ds(off, w)],
                                        v_aug[:, tj, ds((2 * p + 1) * 128, 128)],
                                        e_B[:, ds(half * 512 + off, w)],
                                        start=(tj == 0), stop=(tj == nblk - 1))
                            if s < nsuper:
                                sA = spsum.tile([128, 1024], f32, tag="s",
                                                name=f"sA{p}_{c}_{s}")
                                sB = spsum.tile([128, 1024], f32, tag="s",
                                                name=f"sB{p}_{c}_{s}")
                                i = s - (nsuper - 2)
                                if i < 0:
                                    for half in (0, 1):
                                        tj = 2 * s + half
                                        nc.tensor.matmul(
                                            sA[:, ds(half * 512, 512)],
                                            qk[0:D, p, 1, ts(tj, 128)],
                                            qk[0:D, p, 0, ds(c * 512, 512)],
                                            start=True, stop=True)
                                        nc.tensor.matmul(
                                            sB[:, ds(half * 512, 512)],
                                            qk[D:128, p, 1, ts(tj, 128)],
                                            qk[D:128, p, 0, ds(c * 512, 512)],
                                            start=True, stop=True)
                                else:
                                    # band: columns [0, 128d) are fully masked
                                    # — compute only the live range
                                    for half in (0, 1):
                                        tj = 2 * s + half
                                        off = 128 * (2 * i + half)
                                        w = 512 - off
                                        nc.tensor.matmul(
                                            sA[:, ds(half * 512 + off, w)],
                                            qk[0:D, p, 1, ts(tj, 128)],
                                            qk[0:D, p, 0, ds(c * 512 + off, w)],
                                            start=True, stop=True)
                                        nc.tensor.matmul(
                                            sB[:, ds(half * 512 + off, w)],
                                            qk[D:128, p, 1, ts(tj, 128)],
                                            qk[D:128, p, 0, ds(c * 512 + off, w)],
                                            start=True, stop=True)
                                e_A = epool.tile([128, 1024], bf16, tag="e",
                                                 name=f"eA{p}_{c}_{s}")
                                e_B = epool.tile([128, 1024], bf16, tag="e",
                                                 name=f"eB{p}_{c}_{s}")
                                if i >= 0:
                                    for half in (0, 1):
                                        off = 128 * (2 * i + half)
                                        w = 512 - off
                                        sl = ds(half * 512 + off, w)
                                        nc.scalar.activation(e_A[:, sl], sA[:, sl],
                                                             Exp, scale=INV_SCALE)
                                        nc.scalar.activation(e_B[:, sl], sB[:, sl],
                                                             Exp, scale=INV_SCALE)
                                        # only the diagonal 128 columns of the
                                        # live range contain masked entries;
                                        # zero them with an in-place triangle
                                        # multiply.
                                        sd = ds(half * 512 + off, 128)
                                        nc.vector.tensor_mul(
                                            e_A[:, sd], e_A[:, sd], mask_t[:, 0:128])
                                        nc.vector.tensor_mul(
                                            e_B[:, sd], e_B[:, sd], mask_t[:, 0:128])
                                else:
                                    nc.scalar.activation(e_A[:], sA[:], Exp,
                                                         scale=INV_SCALE)
                                    nc.scalar.activation(e_B[:], sB[:], Exp,
                                                         scale=INV_SCALE)
                                pend[s] = (e_A, e_B)
                            # pace the backlog across the chunk's superslots;
                            # in the final chunk, hold the last items back so the
                            # closing norm chain isn't queued behind their DVE
                            # copies (emitted after the drain below instead).
                            done_slots += 1
                            cap = len(backlog) - 2 if c == NCH - 1 else len(backlog)
                            want = min(cap, -(-len(backlog) * done_slots
                                              // total_slots))
                            while emitted < want and emitted < len(backlog):
                                backlog[emitted]()
                                emitted += 1
                            if done_slots >= 2 and backlog2:
                                half = max(1, (total_slots - 1) // 2)
                                cap2 = (len(backlog2) - 2 if c == NCH - 1
                                        else len(backlog2))
                                want2 = min(cap2, -(-len(backlog2) * (done_slots - 1)
                                                    // half))
                                while emitted2 < want2 and emitted2 < len(backlog2):
                                    backlog2[emitted2]()
                                    emitted2 += 1
                        # stage av to SBUF with one copy so the PSUM banks free
                        # early; normalize from the staged copy.
                        for head, av in ((0, av_A), (1, av_B)):
                            avs = npool.tile([D + 1, 512], f32, tag="avs",
                                             bufs=4, name=f"avs{p}_{c}_{head}")
                            nc.vector.tensor_copy(avs[:], av[0:D + 1, :])
                            deferred_norm.append(make_norm(
                                avs, head, p, c,
                                qsplit=(p == NPAIR - 1 and c == NCH - 1)))

                for fn in deferred_norm:
                    fn()
                deferred_norm.clear()

                # held-back final-chunk backlog items run after the norm chain
                if leftover is not None:
                    bl, bl2 = leftover
                    for item in bl[emitted:]:
                        item()
                    for item in bl2[emitted2:]:
                        item()

                # ---- projection tail (last query chunk) ----
                for t in range(12, NT):
                    proj_group(t)()

    nc.compile()
    return nc


def _make_mask():
    # mask[p, j] = 1 iff j >= p: causal triangle in the first 128 cols of a
    # band live range, ones beyond.
    p = np.arange(128)[:, None]
    j = np.arange(512)[None, :]
    return (j >= p).astype(ml_dtypes.bfloat16)


def kernel(x: np.ndarray, W_attn: np.ndarray, W_proj: np.ndarray) -> np.ndarray:
    global LAST_RESULTS
    x = np.asarray(x, dtype=np.float32)
    W_attn = np.asarray(W_attn, dtype=np.float32)
    W_proj = np.asarray(W_proj, dtype=np.float32)

    nc = _cache.get("nc")
    if nc is None:
        nc = _build()
        _cache["nc"] = nc

    mask = _make_mask()
    xTs = [np.ascontiguousarray(x[b].T).astype(ml_dtypes.bfloat16) for b in range(B)]
    in_maps = []
    for cid in range(NCORES):
        b, hh = cid // 2, cid % 2
        qcols = W_attn[:, hh * 512:(hh + 1) * 512]
        kcols = W_attn[:, C + hh * 512:C + (hh + 1) * 512]
        wqk = np.concatenate([qcols, kcols], axis=1)                  # [1024, 1024]
        # pack to [p, ko, pair, m2, mm] -> [128, NK, 1024]
        wqk_pack = np.ascontiguousarray(
            wqk.reshape(NK, 128, 2, NPAIR, 128).transpose(1, 0, 3, 2, 4)
            .reshape(128, NK, NPAIR * 256)
        ).astype(ml_dtypes.bfloat16)
        wv = np.ascontiguousarray(
            W_attn[:, 2 * C + hh * 512:2 * C + (hh + 1) * 512]
        ).astype(ml_dtypes.bfloat16)
        wp = np.ascontiguousarray(W_proj[hh * 512:(hh + 1) * 512, :]).astype(np.float16)
        in_maps.append({
            "xT": xTs[b], "wqk": wqk_pack, "wv": wv, "wp": wp, "mask": mask,
        })

    res = run_bass_kernel_spmd(nc, in_maps, core_ids=list(range(NCORES)))
    LAST_RESULTS = res
    parts = [res.results[cid]["out"].astype(np.float32) for cid in range(NCORES)]
    out = np.stack([parts[2 * b] + parts[2 * b + 1] for b in range(B)], axis=0)
    return np.ascontiguousarray(out, dtype=np.float32)
